# revision 1
# baseline (speedup 1.0000x reference)
"""S4D "CopyingModel" Trainium2 Bass kernel.

Math: logits = (s4d_scan(emb[x]) + emb[x]*D) @ W_out + b_out, with a
per-channel diagonal SSM (d_model=1024 channels, d_state=64).

Strategy (8 NeuronCores, channel-sharded: 128 channels per core, every core
sees all 8 batches x 4096 tokens):
  - host precomputes (f64) the discretized per-channel operators and ships
    them as fp16 matmul weights:
      * T[d]  [L,L]  lower-tri Toeplitz of the truncated conv kernel
               K[d,k] = sum_n C*dB*dA^k (skip D folded into K[d,0])
      * E[d]  [L,N]  chunk-end state accumulator dA^(L-1-j)*dB
      * Cb[d] [N,L]  state->output C*dA^(i+1)
      * P[d,n] = dA^L chunk-to-chunk decay
  - embedding gather = one-hot (host-encoded) matmul against emb slice
  - sequence is chunked: L=128, 32 chunks x 8 batches = 256 free columns
    per per-channel matmul; chunk-carry is a 31-step elementwise scan
  - y is transposed back to channel-major with PE transposes, then the
    output projection produces logits^T partials; host sums cores + bias.

All matmuls fp16 operands with fp32 PSUM accumulation (validated ~4e-4
relative error end-to-end vs the f32 reference).
"""

import os
from contextlib import ExitStack

import numpy as np

BATCH = 8
SEQ = 4096
D_MODEL = 1024
N_STATE = 64
VOCAB = 64
L = 128                   # chunk length
NCH = SEQ // L            # 32 chunks
NCORES = 8
DPC = D_MODEL // NCORES   # 128 channels per core
BC = NCH * BATCH          # 256 (chunk, batch) tiles; index t = c*BATCH + b

# DMA batch sizes (tiles per SWDGE transfer)
GA = 8    # one-hot tiles per DMA (phase A)
GT = 4    # T (Toeplitz) channels per DMA
GE = 8    # E channels per DMA
GQ = 4    # Cb channel-pairs per DMA
GO = 8    # logits tiles per output DMA

LAST_RESULTS = None       # BassKernelResults of the most recent run (for test.py)


def _precompute_host(emb, log_neg_A, Bmat, C, Dvec, log_dt, W_out):
    """Float64 host precompute of all device operands."""
    dt = np.exp(log_dt.astype(np.float64))                    # (D,)
    A = -np.exp(log_neg_A.astype(np.float64))                 # (D,N)
    dA = np.exp(dt[:, None] * A)                              # (D,N)
    dB = (dA - 1.0) / A * Bmat.astype(np.float64)             # (D,N)
    w = C.astype(np.float64) * dB                             # (D,N)

    # dApow[d,n,k] = dA^k, k=0..L-1
    dApow = np.ones((D_MODEL, N_STATE, L))
    np.cumprod(np.broadcast_to(dA[:, :, None], (D_MODEL, N_STATE, L - 1)),
               axis=2, out=dApow[:, :, 1:])
    K = np.einsum("dn,dnk->dk", w, dApow)                     # (D,L)
    K[:, 0] += Dvec.astype(np.float64)                        # fold skip

    # Toeplitz lhsT: T[d][j,i] = K[d, i-j] for i>=j
    T = np.zeros((D_MODEL, L, L), np.float32)
    Kf = K.astype(np.float32)
    for k in range(L):
        idx = np.arange(L - k)
        T[:, idx, idx + k] = Kf[:, k][:, None]

    # E lhsT [d, j, n] = dA^(L-1-j) * dB
    E = (dApow[:, :, ::-1] * dB[:, :, None]).transpose(0, 2, 1)  # (D,L,N)
    # Cb lhsT [d, n, i] = C * dA^(i+1)
    dApow1 = dApow * dA[:, :, None]
    Cb = C.astype(np.float64)[:, :, None] * dApow1               # (D,N,L)
    P = dApow1[:, :, L - 1]                                      # dA^L (D,N)
    return (T.astype(np.float16), E.astype(np.float16),
            Cb.astype(np.float16), P)


def _emit_kernel(nc, tile, mybir, make_identity):
    f16 = mybir.dt.float16
    f32 = mybir.dt.float32

    # DRAM inputs are host-packed so every DMA is a large contiguous copy
    # landing directly in the SBUF tile layout.
    onehotT = nc.dram_tensor("onehot_t", [VOCAB, BC * L], f16,
                             kind="ExternalInput").ap()
    embs = nc.dram_tensor("emb_s", [VOCAB, DPC], f16, kind="ExternalInput").ap()
    # [group, j, ch_in_group, i]
    t_all = nc.dram_tensor("t_all", [DPC // GT, L, GT, L], f16,
                           kind="ExternalInput").ap()
    e_all = nc.dram_tensor("e_all", [DPC // GE, L, GE, N_STATE], f16,
                           kind="ExternalInput").ap()
    # [group, p=(par,n), q_in_group, i]
    cb_all = nc.dram_tensor("cb_all", [64 // GQ, 128, GQ, L], f16,
                            kind="ExternalInput").ap()
    pdecay = nc.dram_tensor("pdecay", [128, 64, BATCH], f16,
                            kind="ExternalInput").ap()
    w2 = nc.dram_tensor("w2", [DPC, VOCAB], f16, kind="ExternalInput").ap()
    out_t = nc.dram_tensor("out_t", [VOCAB, BC * L], f32,
                           kind="ExternalOutput").ap()

    with tile.TileContext(nc) as tc, ExitStack() as ctx:
        persist = ctx.enter_context(tc.tile_pool(name="persist", bufs=1))
        u_sb = persist.tile([128, BC, DPC], f16, name="u_sb")    # [j, t, dl]
        y_sb = persist.tile([128, DPC, BC], f16, name="y_sb")    # [j, dl, t]
        # x_sb: [(par,n), q, s, b]; slot s=0 is the zero initial state,
        # expand writes S[c] into s=c+1; after the scan, slot s=c holds
        # hstart[c] (state at the beginning of chunk c).
        x_sb = persist.tile([128, 64, NCH + 1, BATCH], f16, name="x_sb")
        p_sb = persist.tile([128, 64, BATCH], f16, name="p_sb")
        emb_sb = persist.tile([VOCAB, DPC], f16, name="emb_sb")
        w2_sb = persist.tile([DPC, VOCAB], f16, name="w2_sb")
        ident = persist.tile([128, 128], f16, name="ident")

        make_identity(nc, ident)
        nc.gpsimd.dma_start(out=emb_sb, in_=embs)
        nc.gpsimd.dma_start(out=w2_sb, in_=w2)
        nc.gpsimd.dma_start(out=p_sb, in_=pdecay)
        nc.vector.memset(x_sb[:, :, 0, :], 0.0)

        def cp(i, out, in_):
            if i % 2 == 0:
                nc.vector.tensor_copy(out, in_)
            else:
                nc.scalar.copy(out, in_)

        # ---- Phase A: embedding (one-hot @ emb slice) -> u_sb ----
        with tc.tile_pool(name="ohp", bufs=3) as ohp, \
             tc.tile_pool(name="ps_a", bufs=8, space="PSUM") as ps_a:
            for g in range(BC // GA):
                oh = ohp.tile([VOCAB, GA, L], f16)
                nc.gpsimd.dma_start(
                    out=oh, in_=onehotT[:, g * GA * L:(g + 1) * GA * L])
                for i in range(GA):
                    t = g * GA + i
                    ups = ps_a.tile([128, DPC], f32)
                    nc.tensor.matmul(ups, lhsT=oh[:, i, :], rhs=emb_sb,
                                     start=True, stop=True)
                    cp(t, u_sb[:, t, :], ups)

        # ---- Phase B: chunk-end states S[c] -> x_sb slots 1..NCH ----
        with tc.tile_pool(name="ewp", bufs=3) as ewp, \
             tc.tile_pool(name="ps_s", bufs=4, space="PSUM") as ps_s:
            for g in range(DPC // GE):
                e_w = ewp.tile([L, GE, N_STATE], f16)
                nc.gpsimd.dma_start(out=e_w, in_=e_all[g])
                for i in range(GE // 2):
                    q = (g * GE) // 2 + i
                    s_ps = ps_s.tile([128, NCH, BATCH], f32)
                    for par in range(2):
                        dl = 2 * q + par
                        nc.tensor.matmul(
                            s_ps[64 * par:64 * (par + 1), :, :],
                            lhsT=e_w[:, 2 * i + par, :], rhs=u_sb[:, :, dl],
                            start=True, stop=True,
                            tile_position=(0, 64 * par) if par else None)
                    cp(q, x_sb[:, q, 1:, :], s_ps)

        # ---- Phase C: chunk-carry scan (31 steps) ----
        with tc.tile_pool(name="scp", bufs=2) as scp:
            for c in range(1, NCH):
                tmp = scp.tile([128, 64, BATCH], f16)
                nc.vector.tensor_mul(tmp, x_sb[:, :, c - 1, :], p_sb)
                nc.vector.tensor_add(x_sb[:, :, c, :], tmp, x_sb[:, :, c, :])

        # ---- Phase D: per-channel y = T^T u (+) Cb^T hstart -> y_sb ----
        with tc.tile_pool(name="twp", bufs=3) as twp, \
             tc.tile_pool(name="cbp", bufs=3) as cbp, \
             tc.tile_pool(name="ps_y", bufs=6, space="PSUM") as ps_y:
            for g in range(64 // GQ):          # loop over Cb groups (4 pairs)
                cb_w = cbp.tile([128, GQ, L], f16)
                nc.gpsimd.dma_start(out=cb_w, in_=cb_all[g])
                for i in range(GQ):
                    q = g * GQ + i
                    if q % (GT // 2) == 0:
                        t_w = twp.tile([L, GT, L], f16)
                        nc.gpsimd.dma_start(out=t_w, in_=t_all[(2 * q) // GT])
                    y_pair = []
                    for par in range(2):
                        dl = 2 * q + par
                        y_ps = ps_y.tile([L, BC], f32)
                        nc.tensor.matmul(y_ps, lhsT=t_w[:, (dl % GT), :],
                                         rhs=u_sb[:, :, dl],
                                         start=True, stop=False)
                        y_pair.append(y_ps)
                    for par in range(2):
                        h = x_sb[64 * par:64 * (par + 1), q, 0:NCH, :]
                        nc.tensor.matmul(
                            y_pair[par],
                            lhsT=cb_w[64 * par:64 * (par + 1), i, :],
                            rhs=h, start=False, stop=True,
                            tile_position=(64 * par, 0) if par else None)
                    for par in range(2):
                        dl = 2 * q + par
                        cp(dl, y_sb[:, dl, :], y_pair[par])

        # ---- Phase E: transpose y + output projection -> out_t ----
        with tc.tile_pool(name="ytp", bufs=4) as ytp, \
             tc.tile_pool(name="lop", bufs=2) as lop, \
             tc.tile_pool(name="ps_t", bufs=2, space="PSUM") as ps_t, \
             tc.tile_pool(name="ps_o", bufs=2, space="PSUM") as ps_o:
            for g in range(BC // GO):
                lo = lop.tile([VOCAB, GO, L], f32)
                for i in range(GO):
                    t = g * GO + i
                    tp = ps_t.tile([128, 128], f16)
                    nc.tensor.transpose(tp, y_sb[:, :, t], ident)
                    yt = ytp.tile([128, 128], f16)
                    cp(t, yt, tp)
                    op = ps_o.tile([VOCAB, L], f32)
                    nc.tensor.matmul(op, lhsT=w2_sb, rhs=yt,
                                     start=True, stop=True)
                    cp(t + 1, lo[:, i, :], op)
                nc.gpsimd.dma_start(
                    out=out_t[:, g * GO * L:(g + 1) * GO * L], in_=lo)


def _build_nc():
    import concourse.tile as tile
    from concourse import bacc, mybir
    from concourse.masks import make_identity

    nc = bacc.Bacc(trn_type="TRN2", target_bir_lowering=False, debug=False)
    _emit_kernel(nc, tile, mybir, make_identity)
    nc.compile()
    return nc


_NC_CACHE = None


def kernel(x, emb, log_neg_A, B, C, D, log_dt, W_out, b_out):
    global LAST_RESULTS, _NC_CACHE
    from concourse.bass_utils import run_bass_kernel_spmd

    x = np.asarray(x).astype(np.int64)
    emb = np.asarray(emb, np.float32)
    log_neg_A = np.asarray(log_neg_A, np.float32)
    B_in = np.asarray(B, np.float32)
    C = np.asarray(C, np.float32)
    D_in = np.asarray(D, np.float32)
    log_dt = np.asarray(log_dt, np.float32)
    W_out = np.asarray(W_out, np.float32)
    b_out = np.asarray(b_out, np.float32)

    T, E, Cb, P = _precompute_host(emb, log_neg_A, B_in, C, D_in, log_dt, W_out)

    # one-hot, token order tok = (c*BATCH + b)*L + j
    toks = x.reshape(BATCH, NCH, L).transpose(1, 0, 2).reshape(-1)
    onehotT = (np.arange(VOCAB)[:, None] == toks[None, :]).astype(np.float16)

    in_maps = []
    for core in range(NCORES):
        ds = slice(core * DPC, (core + 1) * DPC)
        # pdecay layout [p=(par,n), q, b]: p = 64*par + n, d = 2*q + par
        Pc = P[ds].reshape(64, 2, N_STATE).transpose(1, 2, 0).reshape(128, 64)
        Pc = np.ascontiguousarray(
            np.broadcast_to(Pc[:, :, None], (128, 64, BATCH))).astype(np.float16)
        # t_all: [DPC,L,L] -> [DPC/GT, L, GT, L] (ch groups, j-major)
        Tc = np.ascontiguousarray(
            T[ds].reshape(DPC // GT, GT, L, L).transpose(0, 2, 1, 3))
        # e_all: [DPC,L,N] -> [DPC/GE, L, GE, N]
        Ec = np.ascontiguousarray(
            E[ds].reshape(DPC // GE, GE, L, N_STATE).transpose(0, 2, 1, 3))
        # cb_all: [DPC,N,L] -> pair-pack [64, 128=(par,n), L] -> groups of GQ
        Cbp = Cb[ds].reshape(64, 2 * N_STATE, L)   # [q, (par,n), L]
        Cbc = np.ascontiguousarray(
            Cbp.reshape(64 // GQ, GQ, 128, L).transpose(0, 2, 1, 3))
        in_maps.append({
            "onehot_t": onehotT,
            "emb_s": np.ascontiguousarray(emb[:, ds]).astype(np.float16),
            "t_all": Tc,
            "e_all": Ec,
            "cb_all": Cbc,
            "pdecay": Pc,
            "w2": np.ascontiguousarray(W_out[ds]).astype(np.float16),
        })

    if _NC_CACHE is None:
        _NC_CACHE = _build_nc()
    nc = _NC_CACHE

    trace = bool(int(os.environ.get("BASS_TRACE", "0") or "0"))
    LAST_RESULTS = run_bass_kernel_spmd(
        nc, in_maps, core_ids=list(range(NCORES)), trace=trace)

    logitsT = np.zeros((VOCAB, BC * L), np.float64)
    for r in LAST_RESULTS.results:
        logitsT += r["out_t"].astype(np.float64)
    out = (logitsT.T.reshape(NCH, BATCH, L, VOCAB)
           .transpose(1, 0, 2, 3).reshape(BATCH, SEQ, VOCAB))
    return (out + b_out.astype(np.float64)).astype(np.float32)



# revision 12
# speedup vs baseline: 1.0383x; 1.0383x over previous
"""S4D "CopyingModel" Trainium2 Bass kernel (v2 — restructured pipeline).

Math: logits = (s4d_scan(emb[x]) + emb[x]*D) @ W_out + b_out, with a
per-channel diagonal SSM (d_model=1024 channels, d_state=64).

Strategy (8 NeuronCores, channel-sharded: 128 channels per core, every core
sees all 8 batches x 4096 tokens):
  - host precomputes (f64) the discretized per-channel operators and ships
    them as fp16 matmul weights:
      * T[d]  [L,L]  lower-tri Toeplitz of the truncated conv kernel
               K[d,k] = sum_n C*dB*dA^k (skip D folded into K[d,0])
      * E[d]  [L,N]  chunk-end state accumulator dA^(L-1-j)*dB
      * Cb[d] [N,L]  state->output C*dA^(i+1)
      * pmul  [p,q,c] chunk-to-chunk decay dA^L (0 at c=0 to reset chains)
  - embedding gather = one-hot (host-encoded) matmul against emb slice
  - sequence chunked: L=128, 32 chunks x 8 batches = 256 free columns
    per per-channel matmul
  - chunk-carry handled by hardware tensor_tensor_scan (fp32 state) split
    across DVE + Pool, overlapped with the Toeplitz matmuls
  - y tiles alias u's SBUF storage (channel column overwritten after its
    last read); output projection consumes y via DMA XBAR transposes, so
    no PE transposes and no extra SBUF copies
  - logits^T partials written as fp16; host sums cores + bias.
"""

import os
from contextlib import ExitStack

import numpy as np

BATCH = 8
SEQ = 4096
D_MODEL = 1024
N_STATE = 64
VOCAB = 64
L = 128                   # chunk length
NCH = SEQ // L            # 32 chunks
NCORES = 8
DPC = D_MODEL // NCORES   # 128 channels per core
BC = NCH * BATCH          # 256 (chunk, batch) tiles; index t = c*BATCH + b

# DMA batch sizes
GA = 16   # one-hot tiles per DMA (phase A)
GT = 16   # T (Toeplitz) channels per DMA
GE = 16   # E channels per DMA
GQ = 8    # Cb channel-pairs per DMA
AHEAD = 6 # T-matmul channel-pairs in flight ahead of Cb in phase D

LAST_RESULTS = None       # BassKernelResults of the most recent run (for test.py)


def _precompute_host(emb, log_neg_A, Bmat, C, Dvec, log_dt, W_out):
    """Float64 host precompute of all device operands."""
    dt = np.exp(log_dt.astype(np.float64))                    # (D,)
    A = -np.exp(log_neg_A.astype(np.float64))                 # (D,N)
    dA = np.exp(dt[:, None] * A)                              # (D,N)
    dB = (dA - 1.0) / A * Bmat.astype(np.float64)             # (D,N)
    w = C.astype(np.float64) * dB                             # (D,N)

    # dApow[d,n,k] = dA^k, k=0..L-1
    dApow = np.ones((D_MODEL, N_STATE, L))
    np.cumprod(np.broadcast_to(dA[:, :, None], (D_MODEL, N_STATE, L - 1)),
               axis=2, out=dApow[:, :, 1:])
    K = np.einsum("dn,dnk->dk", w, dApow)                     # (D,L)
    K[:, 0] += Dvec.astype(np.float64)                        # fold skip

    # Toeplitz lhsT: T[d][j,i] = K[d, i-j] for i>=j
    T = np.zeros((D_MODEL, L, L), np.float32)
    Kf = K.astype(np.float32)
    for k in range(L):
        idx = np.arange(L - k)
        T[:, idx, idx + k] = Kf[:, k][:, None]

    # E lhsT [d, j, n] = dA^(L-1-j) * dB
    E = (dApow[:, :, ::-1] * dB[:, :, None]).transpose(0, 2, 1)  # (D,L,N)
    # Cb lhsT [d, n, i] = C * dA^(i+1)
    dApow1 = dApow * dA[:, :, None]
    Cb = C.astype(np.float64)[:, :, None] * dApow1               # (D,N,L)
    P = dApow1[:, :, L - 1]                                      # dA^L (D,N)
    return (T.astype(np.float16), E.astype(np.float16),
            Cb.astype(np.float16), P)


def _emit_kernel(nc, tile, mybir, make_identity):
    f16 = mybir.dt.float16
    f32 = mybir.dt.float32
    mult = mybir.AluOpType.mult
    add = mybir.AluOpType.add

    onehotT = nc.dram_tensor("onehot_t", [VOCAB, BC * L], f16,
                             kind="ExternalInput").ap()
    embs = nc.dram_tensor("emb_s", [VOCAB, DPC], f16, kind="ExternalInput").ap()
    # [group, j, ch_in_group, i]
    t_all = nc.dram_tensor("t_all", [DPC // GT, L, GT, L], f16,
                           kind="ExternalInput").ap()
    e_all = nc.dram_tensor("e_all", [DPC // GE, L, GE, N_STATE], f16,
                           kind="ExternalInput").ap()
    # [group, p=(par,n), ch_in_group, i] — Cb zero-padded to K=128
    cb_all = nc.dram_tensor("cb_all", [DPC // GQ, 128, GQ, L], f16,
                            kind="ExternalInput").ap()
    # [p=(par,n), q, c]: dA^L, zeroed at c=0 (scan-chain reset)
    pmul = nc.dram_tensor("pmul", [128, 64, NCH], f16,
                          kind="ExternalInput").ap()
    w2 = nc.dram_tensor("w2", [DPC, VOCAB], f16, kind="ExternalInput").ap()
    out_t = nc.dram_tensor("out_t", [VOCAB, BC * L], f16,
                           kind="ExternalOutput").ap()

    with tile.TileContext(nc) as tc, ExitStack() as ctx:
        persist = ctx.enter_context(tc.tile_pool(name="persist", bufs=1))
        # u_sb: [j, t, dl]; after phase D each channel column dl is
        # overwritten with y (tile t then holds y^T-input [i, dl]).
        u_sb = persist.tile([128, BC, DPC], f16, name="u_sb")
        # xin: [p=(par,n), b, q, c]; slot c holds S[c-1] (c=0 is zero);
        # tensor_tensor_scan rewrites slot c with hstart[c] in place.
        xin = persist.tile([128, BATCH, 64, NCH], f16, name="xin")
        pm_sb = persist.tile([128, 64, NCH], f16, name="pm_sb")
        emb_sb = persist.tile([VOCAB, DPC], f16, name="emb_sb")
        w2_sb = persist.tile([DPC, VOCAB], f16, name="w2_sb")
        ident = persist.tile([128, 128], f16, name="ident")

        make_identity(nc, ident)
        nc.gpsimd.dma_start(out=emb_sb, in_=embs)
        nc.gpsimd.dma_start(out=w2_sb, in_=w2)
        nc.gpsimd.dma_start(out=pm_sb, in_=pmul)
        nc.vector.memset(xin[:, :, :, 0], 0.0)

        cp_engines = [nc.vector, nc.scalar, nc.gpsimd]

        def cp(k, out, in_, n=2):
            e = cp_engines[k % n]
            if e is nc.scalar:
                e.copy(out, in_)
            else:
                e.tensor_copy(out, in_)

        # ---- Phase A: embedding (one-hot @ emb slice) -> u_sb ----
        with tc.tile_pool(name="ohp", bufs=3) as ohp, \
             tc.tile_pool(name="ps_a", bufs=3, space="PSUM") as ps_a:
            for g in range(BC // GA):
                oh = ohp.tile([VOCAB, GA, L], f16, name="oh")
                nc.gpsimd.dma_start(
                    out=oh, in_=onehotT[:, g * GA * L:(g + 1) * GA * L])
                for h in range(GA // 4):
                    ups = ps_a.tile([128, 4, DPC], f32, name="ups")
                    for i in range(4):
                        nc.tensor.matmul(ups[:, i, :], lhsT=oh[:, h * 4 + i, :],
                                         rhs=emb_sb, start=True, stop=True)
                    t0 = g * GA + h * 4
                    cp(g * (GA // 4) + h, u_sb[:, t0:t0 + 4, :], ups, n=2)

        # ---- Phase B: chunk-end states S[c] -> xin slots 1..31 ----
        # ---- Phase C: hardware scan (fp32 state) -> hstart in place ----
        def emit_scans(half):
            sl = slice(32 * half, 32 * (half + 1))
            for b in range(BATCH):
                ap = xin[:, b, sl, :].opt()
                nc.vector.tensor_tensor_scan(out=ap, data0=pm_sb[:, sl, :].opt(),
                                       data1=ap, initial=0.0,
                                       op0=mult, op1=add)

        with tc.tile_pool(name="ewp", bufs=2) as ewp, \
             tc.tile_pool(name="ps_s", bufs=4, space="PSUM") as ps_s:
            e_w = None
            for q in range(64):
                if q % (GE // 2) == 0:
                    e_w = ewp.tile([L, GE, N_STATE], f16, name="e_w")
                    nc.gpsimd.dma_start(out=e_w, in_=e_all[q // (GE // 2)])
                s_ps = ps_s.tile([128, NCH, BATCH], f32, name="s_ps")
                for par in range(2):
                    nc.tensor.matmul(
                        s_ps[64 * par:64 * (par + 1), :, :],
                        lhsT=e_w[:, 2 * (q % (GE // 2)) + par, :],
                        rhs=u_sb[:, :, 2 * q + par],
                        start=True, stop=True,
                        tile_position=(0, 64 * par) if par else None)
                # S[0..30] -> xin[:, :, q, 1:32] in (c, b) iteration order
                dst = xin[:, :, q, 1:].transpose([0, 2, 1])
                nc.scalar.copy(dst, s_ps[:, 0:NCH - 1, :])
                if q == 31:
                    emit_scans(0)
            emit_scans(1)

        # ---- Phase D: y = T^T u (+) Cb^T hstart -> y columns in u_sb ----
        with tc.tile_pool(name="twp", bufs=2) as twp, \
             tc.tile_pool(name="cbp", bufs=2) as cbp, \
             tc.tile_pool(name="ps_y", bufs=AHEAD, space="PSUM") as ps_y:
            t_w = [None]
            cb_w = [None]
            tps = {}

            def emit_T(q):
                if q % (GT // 2) == 0:
                    t_w[0] = twp.tile([L, GT, L], f16, name="t_w")
                    nc.gpsimd.dma_start(out=t_w[0], in_=t_all[q // (GT // 2)])
                y_ps = ps_y.tile([L, 2, BC], f32, name="y_ps")
                for par in range(2):
                    dl = 2 * q + par
                    nc.tensor.matmul(y_ps[:, par, :], lhsT=t_w[0][:, dl % GT, :],
                                     rhs=u_sb[:, :, dl], start=(par == 0),
                                     stop=False)
                tps[q] = y_ps

            def emit_cb(q):
                if (2 * q) % GQ == 0:
                    cb_w[0] = cbp.tile([128, GQ, L], f16, name="cb_w")
                    nc.gpsimd.dma_start(out=cb_w[0], in_=cb_all[(2 * q) // GQ])
                h = xin[:, :, q, :].transpose([0, 2, 1])
                for par in range(2):
                    dl = 2 * q + par
                    nc.tensor.matmul(
                        tps[q][:, par, :],
                        lhsT=cb_w[0][:, dl % GQ, :],
                        rhs=h, start=False, stop=(par == 1))

            for q in range(AHEAD):
                emit_T(q)
            for q in range(64):
                emit_cb(q)
                for par in range(2):
                    dl = 2 * q + par
                    cp(2 * q + par + 1, u_sb[:, :, dl], tps[q][:, par, :], n=2)
                del tps[q]
                if q + AHEAD < 64:
                    emit_T(q + AHEAD)

        # ---- Phase E: XBAR-transpose y tiles + output projection ----
        with tc.tile_pool(name="ytp", bufs=6) as ytp, \
             tc.tile_pool(name="lop", bufs=4) as lop, \
             tc.tile_pool(name="ps_t", bufs=3, space="PSUM") as ps_t, \
             tc.tile_pool(name="ps_o", bufs=3, space="PSUM") as ps_o:
            for g in range(BC // 4):
                lo_ps = ps_o.tile([128, 2, L], f32, name="lo_ps")
                for h in range(2):
                    tp = ps_t.tile([128, 2, L], f16, name="tp")
                    yt = ytp.tile([128, 2, L], f16, name="yt")
                    for i in range(2):
                        t = g * 4 + h * 2 + i
                        nc.tensor.transpose(tp[:, i, :], u_sb[:, t, :], ident)
                    cp(g * 2 + h, yt, tp, n=2)
                    nc.tensor.matmul(lo_ps[64 * h:64 * (h + 1), :, :],
                                     lhsT=w2_sb, rhs=yt,
                                     start=True, stop=True,
                                     tile_position=(0, 64 * h) if h else None)
                lo = lop.tile([128, 2, L], f16, name="lo")
                cp(g, lo, lo_ps, n=2)
                base = g * 4 * L
                nc.gpsimd.dma_start(out=out_t[:, base:base + 2 * L],
                                    in_=lo[0:64, :, :])
                nc.gpsimd.dma_start(out=out_t[:, base + 2 * L:base + 4 * L],
                                    in_=lo[64:128, :, :])


def _build_nc():
    import concourse.tile as tile
    from concourse import bacc, mybir

    from concourse.masks import make_identity
    nc = bacc.Bacc(trn_type="TRN2", target_bir_lowering=False, debug=False)
    _emit_kernel(nc, tile, mybir, make_identity)
    nc.compile()
    return nc


_NC_CACHE = None


def kernel(x, emb, log_neg_A, B, C, D, log_dt, W_out, b_out):
    global LAST_RESULTS, _NC_CACHE
    from concourse.bass_utils import run_bass_kernel_spmd

    x = np.asarray(x).astype(np.int64)
    emb = np.asarray(emb, np.float32)
    log_neg_A = np.asarray(log_neg_A, np.float32)
    B_in = np.asarray(B, np.float32)
    C = np.asarray(C, np.float32)
    D_in = np.asarray(D, np.float32)
    log_dt = np.asarray(log_dt, np.float32)
    W_out = np.asarray(W_out, np.float32)
    b_out = np.asarray(b_out, np.float32)

    T, E, Cb, P = _precompute_host(emb, log_neg_A, B_in, C, D_in, log_dt, W_out)

    # one-hot, token order tok = (c*BATCH + b)*L + j
    toks = x.reshape(BATCH, NCH, L).transpose(1, 0, 2).reshape(-1)
    onehotT = (np.arange(VOCAB)[:, None] == toks[None, :]).astype(np.float16)

    in_maps = []
    for core in range(NCORES):
        ds = slice(core * DPC, (core + 1) * DPC)
        # pmul layout [p=(par,n), q, c]: p = 64*par + n, d = 2*q + par
        Pc = P[ds].reshape(64, 2, N_STATE).transpose(1, 2, 0).reshape(128, 64)
        pm = np.broadcast_to(Pc[:, :, None], (128, 64, NCH)).copy()
        pm[:, :, 0] = 0.0
        # t_all: [DPC,L,L] -> [DPC/GT, L, GT, L] (ch groups, j-major)
        Tc = np.ascontiguousarray(
            T[ds].reshape(DPC // GT, GT, L, L).transpose(0, 2, 1, 3))
        # e_all: [DPC,L,N] -> [DPC/GE, L, GE, N]
        Ec = np.ascontiguousarray(
            E[ds].reshape(DPC // GE, GE, L, N_STATE).transpose(0, 2, 1, 3))
        # cb_all: [DPC,N,L] zero-padded to [DPC,128,L]: rows 64*(d%2)+n
        Cbpad = np.zeros((DPC, 128, L), np.float16)
        for par in range(2):
            Cbpad[par::2, 64 * par:64 * (par + 1), :] = Cb[ds][par::2]
        Cbc = np.ascontiguousarray(
            Cbpad.reshape(DPC // GQ, GQ, 128, L).transpose(0, 2, 1, 3))
        in_maps.append({
            "onehot_t": onehotT,
            "emb_s": np.ascontiguousarray(emb[:, ds]).astype(np.float16),
            "t_all": Tc,
            "e_all": Ec,
            "cb_all": Cbc,
            "pmul": pm.astype(np.float16),
            "w2": np.ascontiguousarray(W_out[ds]).astype(np.float16),
        })

    if _NC_CACHE is None:
        _NC_CACHE = _build_nc()
    nc = _NC_CACHE

    trace = bool(int(os.environ.get("BASS_TRACE", "0") or "0"))
    LAST_RESULTS = run_bass_kernel_spmd(
        nc, in_maps, core_ids=list(range(NCORES)), trace=trace)

    logitsT = np.zeros((VOCAB, BC * L), np.float64)
    for r in LAST_RESULTS.results:
        logitsT += r["out_t"].astype(np.float64)
    out = (logitsT.T.reshape(NCH, BATCH, L, VOCAB)
           .transpose(1, 0, 2, 3).reshape(BATCH, SEQ, VOCAB))
    return (out + b_out.astype(np.float64)).astype(np.float32)


# revision 13
# speedup vs baseline: 1.5928x; 1.5340x over previous
"""S4D "CopyingModel" Trainium2 Bass kernel (v4 — contiguous-copy pipeline).

Math: logits = (s4d_scan(emb[x]) + emb[x]*D) @ W_out + b_out, with a
per-channel diagonal SSM (d_model=1024 channels, d_state=64).

Strategy (8 NeuronCores, channel-sharded: 128 channels per core, every core
sees all 8 batches x 4096 tokens):
  - host precomputes (f64) the discretized per-channel operators and ships
    them as fp16 matmul weights:
      * T[d]  [L,L]  lower-tri Toeplitz of the truncated conv kernel
               K[d,k] = sum_n C*dB*dA^k (skip D folded into K[d,0])
      * E[d]  [L,N]  chunk-end state accumulator dA^(L-1-j)*dB
      * Cb[d] [N,L]  state->output C*dA^(i+1), zero-padded to K=128 rows
      * pmul  [p,q,c] chunk-to-chunk decay dA^L (0 at c=0 to reset chains)
  - u = emb[x] gathered on host, shipped per-core as [j, dl, t] fp16 so
    every matmul rhs read and every engine copy is contiguous
  - sequence chunked: L=128, tiles t = b*32 + c (batch-major)
  - chunk-carry handled by hardware tensor_tensor_scan (fp32 state) on DVE,
    overlapped with the Toeplitz matmuls
  - output projection via PE transposes + fp16 logits^T partials;
    host sums cores + bias.
"""

import os
from contextlib import ExitStack

import numpy as np

BATCH = 8
SEQ = 4096
D_MODEL = 1024
N_STATE = 64
VOCAB = 64
L = 128                   # chunk length
NCH = SEQ // L            # 32 chunks
NCORES = 8
DPC = D_MODEL // NCORES   # 128 channels per core
BC = NCH * BATCH          # 256 (batch, chunk) tiles; index t = b*NCH + c

# DMA batch sizes
GU = 16   # u channel-columns per DMA
GT = 16   # T (Toeplitz) channels per DMA
GE = 16   # E channels per DMA
GQ = 8    # Cb channels per DMA
AHEAD = 6 # T-matmul channel-pairs in flight ahead of Cb in phase D

LAST_RESULTS = None       # BassKernelResults of the most recent run (for test.py)


def _precompute_host(emb, log_neg_A, Bmat, C, Dvec, log_dt, W_out):
    """Float64 host precompute of all device operands."""
    dt = np.exp(log_dt.astype(np.float64))                    # (D,)
    A = -np.exp(log_neg_A.astype(np.float64))                 # (D,N)
    dA = np.exp(dt[:, None] * A)                              # (D,N)
    dB = (dA - 1.0) / A * Bmat.astype(np.float64)             # (D,N)
    w = C.astype(np.float64) * dB                             # (D,N)

    # dApow[d,n,k] = dA^k, k=0..L-1
    dApow = np.ones((D_MODEL, N_STATE, L))
    np.cumprod(np.broadcast_to(dA[:, :, None], (D_MODEL, N_STATE, L - 1)),
               axis=2, out=dApow[:, :, 1:])
    K = np.einsum("dn,dnk->dk", w, dApow)                     # (D,L)
    K[:, 0] += Dvec.astype(np.float64)                        # fold skip

    # Toeplitz lhsT: T[d][j,i] = K[d, i-j] for i>=j
    T = np.zeros((D_MODEL, L, L), np.float32)
    Kf = K.astype(np.float32)
    for k in range(L):
        idx = np.arange(L - k)
        T[:, idx, idx + k] = Kf[:, k][:, None]

    # E lhsT [d, j, n] = dA^(L-1-j) * dB
    E = (dApow[:, :, ::-1] * dB[:, :, None]).transpose(0, 2, 1)  # (D,L,N)
    # Cb lhsT [d, n, i] = C * dA^(i+1)
    dApow1 = dApow * dA[:, :, None]
    Cb = C.astype(np.float64)[:, :, None] * dApow1               # (D,N,L)
    P = dApow1[:, :, L - 1]                                      # dA^L (D,N)
    return (T.astype(np.float16), E.astype(np.float16),
            Cb.astype(np.float16), P)


def _emit_kernel(nc, tile, mybir, make_identity):
    f16 = mybir.dt.float16
    f32 = mybir.dt.float32
    mult = mybir.AluOpType.mult
    add = mybir.AluOpType.add

    # u pre-gathered on host: [j, dl, t] with t = b*NCH + c
    u_in = nc.dram_tensor("u_in", [L, DPC, BC], f16, kind="ExternalInput").ap()
    # [group, j, ch_in_group, i]
    t_all = nc.dram_tensor("t_all", [DPC // GT, L, GT, L], f16,
                           kind="ExternalInput").ap()
    e_all = nc.dram_tensor("e_all", [DPC // GE, L, GE, N_STATE], f16,
                           kind="ExternalInput").ap()
    # [group, p=(par,n), ch_in_group, i] — Cb zero-padded to K=128
    cb_all = nc.dram_tensor("cb_all", [DPC // GQ, 128, GQ, L], f16,
                            kind="ExternalInput").ap()
    # [p=(par,n), q, c]: dA^L, zeroed at c=0 (scan-chain reset)
    pmul = nc.dram_tensor("pmul", [128, 64, NCH], f16,
                          kind="ExternalInput").ap()
    w2 = nc.dram_tensor("w2", [DPC, VOCAB], f16, kind="ExternalInput").ap()
    out_t = nc.dram_tensor("out_t", [VOCAB, BC * L], f16,
                           kind="ExternalOutput").ap()

    with tile.TileContext(nc) as tc, ExitStack() as ctx:
        persist = ctx.enter_context(tc.tile_pool(name="persist", bufs=1))
        u_sb = persist.tile([128, DPC, BC], f16, name="u_sb")    # [j, dl, t]
        y_sb = persist.tile([128, DPC, BC], f16, name="y_sb")    # [i, dl, t]
        # xin: [p=(par,n), b, q, c]; slot c holds S[c-1] (c=0 is zero);
        # tensor_tensor_scan rewrites slot c with hstart[c] in place.
        xin = persist.tile([128, BATCH, 64, NCH], f16, name="xin")
        pm_sb = persist.tile([128, 64, NCH], f16, name="pm_sb")
        w2_sb = persist.tile([DPC, VOCAB], f16, name="w2_sb")
        ident = persist.tile([128, 128], f16, name="ident")

        make_identity(nc, ident)
        nc.gpsimd.dma_start(out=w2_sb, in_=w2)
        nc.gpsimd.dma_start(out=pm_sb, in_=pmul)
        nc.vector.memset(xin[:, :, :, 0], 0.0)

        # ---- Phase A: load u (host-gathered embedding) ----
        for g in range(DPC // GU):
            nc.gpsimd.dma_start(out=u_sb[:, g * GU:(g + 1) * GU, :],
                                in_=u_in[:, g * GU:(g + 1) * GU, :])

        cp_engines = [nc.vector, nc.scalar]

        def cp(k, out, in_):
            e = cp_engines[k % 2]
            if e is nc.scalar:
                e.copy(out, in_)
            else:
                e.tensor_copy(out, in_)

        # ---- Phase B: chunk-end states S[c] -> xin slots 1..31 ----
        # ---- Phase C: hardware scan (fp32 state) -> hstart in place ----
        def emit_scans(half):
            sl = slice(32 * half, 32 * (half + 1))
            for b in range(BATCH):
                ap = xin[:, b, sl, :].opt()
                nc.vector.tensor_tensor_scan(out=ap, data0=pm_sb[:, sl, :].opt(),
                                             data1=ap, initial=0.0,
                                             op0=mult, op1=add)

        with tc.tile_pool(name="ewp", bufs=2) as ewp, \
             tc.tile_pool(name="ps_s", bufs=4, space="PSUM") as ps_s:
            e_w = None
            for q in range(64):
                if q % (GE // 2) == 0:
                    e_w = ewp.tile([L, GE, N_STATE], f16, name="e_w")
                    nc.gpsimd.dma_start(out=e_w, in_=e_all[q // (GE // 2)])
                s_ps = ps_s.tile([128, BATCH, NCH], f32, name="s_ps")
                for par in range(2):
                    nc.tensor.matmul(
                        s_ps[64 * par:64 * (par + 1), :, :],
                        lhsT=e_w[:, 2 * (q % (GE // 2)) + par, :],
                        rhs=u_sb[:, 2 * q + par, :],
                        start=True, stop=True,
                        tile_position=(0, 64 * par) if par else None)
                # S[0..30] -> xin[:, :, q, 1:32]; both iterate (b, c)
                nc.scalar.copy(xin[:, :, q, 1:], s_ps[:, :, 0:NCH - 1])
                if q == 31:
                    emit_scans(0)
            emit_scans(1)

        # ---- Phase D: y = T^T u (+) Cb^T hstart -> y_sb ----
        with tc.tile_pool(name="twp", bufs=2) as twp, \
             tc.tile_pool(name="cbp", bufs=2) as cbp, \
             tc.tile_pool(name="ps_y", bufs=AHEAD, space="PSUM") as ps_y:
            t_w = [None]
            cb_w = [None]
            tps = {}

            def emit_T(q):
                if q % (GT // 2) == 0:
                    t_w[0] = twp.tile([L, GT, L], f16, name="t_w")
                    nc.gpsimd.dma_start(out=t_w[0], in_=t_all[q // (GT // 2)])
                y_ps = ps_y.tile([L, 2, BC], f32, name="y_ps")
                for par in range(2):
                    dl = 2 * q + par
                    nc.tensor.matmul(y_ps[:, par, :], lhsT=t_w[0][:, dl % GT, :],
                                     rhs=u_sb[:, dl, :], start=(par == 0),
                                     stop=False)
                tps[q] = y_ps

            def emit_cb(q):
                if (2 * q) % GQ == 0:
                    cb_w[0] = cbp.tile([128, GQ, L], f16, name="cb_w")
                    nc.gpsimd.dma_start(out=cb_w[0], in_=cb_all[(2 * q) // GQ])
                h = xin[:, :, q, :]
                for par in range(2):
                    dl = 2 * q + par
                    nc.tensor.matmul(
                        tps[q][:, par, :],
                        lhsT=cb_w[0][:, dl % GQ, :],
                        rhs=h, start=False, stop=(par == 1))

            for q in range(AHEAD):
                emit_T(q)
            for q in range(64):
                emit_cb(q)
                cp(q, y_sb[:, 2 * q:2 * q + 2, :], tps[q])
                del tps[q]
                if q + AHEAD < 64:
                    emit_T(q + AHEAD)

        # ---- Phase E: PE-transpose y tiles + output projection ----
        with tc.tile_pool(name="ytp", bufs=6) as ytp, \
             tc.tile_pool(name="lop", bufs=4) as lop, \
             tc.tile_pool(name="ps_t", bufs=3, space="PSUM") as ps_t, \
             tc.tile_pool(name="ps_o", bufs=3, space="PSUM") as ps_o:
            for g in range(BC // 4):
                lo_ps = ps_o.tile([128, 2, L], f32, name="lo_ps")
                for h in range(2):
                    tp = ps_t.tile([128, 2, L], f16, name="tp")
                    yt = ytp.tile([128, 2, L], f16, name="yt")
                    for i in range(2):
                        t = g * 4 + h * 2 + i
                        nc.tensor.transpose(tp[:, i, :], y_sb[:, :, t], ident)
                    cp(g * 2 + h, yt, tp)
                    nc.tensor.matmul(lo_ps[64 * h:64 * (h + 1), :, :],
                                     lhsT=w2_sb, rhs=yt,
                                     start=True, stop=True,
                                     tile_position=(0, 64 * h) if h else None)
                lo = lop.tile([128, 2, L], f16, name="lo")
                cp(g, lo, lo_ps)
                base = g * 4 * L
                nc.gpsimd.dma_start(out=out_t[:, base:base + 2 * L],
                                    in_=lo[0:64, :, :])
                nc.gpsimd.dma_start(out=out_t[:, base + 2 * L:base + 4 * L],
                                    in_=lo[64:128, :, :])


def _build_nc():
    import concourse.tile as tile
    from concourse import bacc, mybir
    from concourse.masks import make_identity

    nc = bacc.Bacc(trn_type="TRN2", target_bir_lowering=False, debug=False)
    _emit_kernel(nc, tile, mybir, make_identity)
    nc.compile()
    return nc


_NC_CACHE = None


def kernel(x, emb, log_neg_A, B, C, D, log_dt, W_out, b_out):
    global LAST_RESULTS, _NC_CACHE
    from concourse.bass_utils import run_bass_kernel_spmd

    x = np.asarray(x).astype(np.int64)
    emb = np.asarray(emb, np.float32)
    log_neg_A = np.asarray(log_neg_A, np.float32)
    B_in = np.asarray(B, np.float32)
    C = np.asarray(C, np.float32)
    D_in = np.asarray(D, np.float32)
    log_dt = np.asarray(log_dt, np.float32)
    W_out = np.asarray(W_out, np.float32)
    b_out = np.asarray(b_out, np.float32)

    T, E, Cb, P = _precompute_host(emb, log_neg_A, B_in, C, D_in, log_dt, W_out)

    # u = emb[x] in fp16; tile order t = b*NCH + c
    emb16 = emb.astype(np.float16)
    u_full = emb16[x].reshape(BATCH, NCH, L, D_MODEL)

    in_maps = []
    for core in range(NCORES):
        ds = slice(core * DPC, (core + 1) * DPC)
        # u_in: [j, dl, t=(b, c)]
        uc = np.ascontiguousarray(
            u_full[:, :, :, ds].transpose(2, 3, 0, 1).reshape(L, DPC, BC))
        # pmul layout [p=(par,n), q, c]: p = 64*par + n, d = 2*q + par
        Pc = P[ds].reshape(64, 2, N_STATE).transpose(1, 2, 0).reshape(128, 64)
        pm = np.broadcast_to(Pc[:, :, None], (128, 64, NCH)).copy()
        pm[:, :, 0] = 0.0
        # t_all: [DPC,L,L] -> [DPC/GT, L, GT, L] (ch groups, j-major)
        Tc = np.ascontiguousarray(
            T[ds].reshape(DPC // GT, GT, L, L).transpose(0, 2, 1, 3))
        # e_all: [DPC,L,N] -> [DPC/GE, L, GE, N]
        Ec = np.ascontiguousarray(
            E[ds].reshape(DPC // GE, GE, L, N_STATE).transpose(0, 2, 1, 3))
        # cb_all: [DPC,N,L] zero-padded to [DPC,128,L]: rows 64*(d%2)+n
        Cbpad = np.zeros((DPC, 128, L), np.float16)
        for par in range(2):
            Cbpad[par::2, 64 * par:64 * (par + 1), :] = Cb[ds][par::2]
        Cbc = np.ascontiguousarray(
            Cbpad.reshape(DPC // GQ, GQ, 128, L).transpose(0, 2, 1, 3))
        in_maps.append({
            "u_in": uc,
            "t_all": Tc,
            "e_all": Ec,
            "cb_all": Cbc,
            "pmul": pm.astype(np.float16),
            "w2": np.ascontiguousarray(W_out[ds]).astype(np.float16),
        })

    if _NC_CACHE is None:
        _NC_CACHE = _build_nc()
    nc = _NC_CACHE

    trace = bool(int(os.environ.get("BASS_TRACE", "0") or "0"))
    LAST_RESULTS = run_bass_kernel_spmd(
        nc, in_maps, core_ids=list(range(NCORES)), trace=trace)

    logitsT = np.zeros((VOCAB, BC * L), np.float64)
    for r in LAST_RESULTS.results:
        logitsT += r["out_t"].astype(np.float64)
    # columns tok = (b*NCH + c)*L + i
    out = logitsT.T.reshape(BATCH, NCH, L, VOCAB).reshape(BATCH, SEQ, VOCAB)
    return (out + b_out.astype(np.float64)).astype(np.float32)


# revision 14
# speedup vs baseline: 1.9342x; 1.2144x over previous
"""S4D "CopyingModel" Trainium2 Bass kernel (v4 — contiguous-copy pipeline).

Math: logits = (s4d_scan(emb[x]) + emb[x]*D) @ W_out + b_out, with a
per-channel diagonal SSM (d_model=1024 channels, d_state=64).

Strategy (8 NeuronCores, channel-sharded: 128 channels per core, every core
sees all 8 batches x 4096 tokens):
  - host precomputes (f64) the discretized per-channel operators and ships
    them as fp16 matmul weights:
      * T[d]  [L,L]  lower-tri Toeplitz of the truncated conv kernel
               K[d,k] = sum_n C*dB*dA^k (skip D folded into K[d,0])
      * E[d]  [L,N]  chunk-end state accumulator dA^(L-1-j)*dB
      * Cb[d] [N,L]  state->output C*dA^(i+1), zero-padded to K=128 rows
      * pmul  [p,q,c] chunk-to-chunk decay dA^L (0 at c=0 to reset chains)
  - u = emb[x] gathered on host, shipped per-core as [j, dl, t] fp16 so
    every matmul rhs read and every engine copy is contiguous
  - sequence chunked: L=128, tiles t = b*32 + c (batch-major)
  - chunk-carry handled by hardware tensor_tensor_scan (fp32 state) on DVE,
    overlapped with the Toeplitz matmuls
  - output projection via PE transposes + fp16 logits^T partials;
    host sums cores + bias.
"""

import os
from contextlib import ExitStack

import numpy as np

BATCH = 8
SEQ = 4096
D_MODEL = 1024
N_STATE = 64
VOCAB = 64
L = 128                   # chunk length
NCH = SEQ // L            # 32 chunks
NCORES = 8
DPC = D_MODEL // NCORES   # 128 channels per core
BC = NCH * BATCH          # 256 (batch, chunk) tiles; index t = b*NCH + c

# DMA batch sizes
GU = 16   # u channel-columns per DMA
GT = 16   # T (Toeplitz) channels per DMA
GE = 16   # E channels per DMA
GQ = 8    # Cb channels per DMA
AHEAD = 6 # T-matmul channel-pairs in flight ahead of Cb in phase D

LAST_RESULTS = None       # BassKernelResults of the most recent run (for test.py)


def _precompute_host(emb, log_neg_A, Bmat, C, Dvec, log_dt, W_out):
    """Float64 host precompute of all device operands."""
    dt = np.exp(log_dt.astype(np.float64))                    # (D,)
    A = -np.exp(log_neg_A.astype(np.float64))                 # (D,N)
    dA = np.exp(dt[:, None] * A)                              # (D,N)
    dB = (dA - 1.0) / A * Bmat.astype(np.float64)             # (D,N)
    w = C.astype(np.float64) * dB                             # (D,N)

    # dApow[d,n,k] = dA^k, k=0..L-1
    dApow = np.ones((D_MODEL, N_STATE, L))
    np.cumprod(np.broadcast_to(dA[:, :, None], (D_MODEL, N_STATE, L - 1)),
               axis=2, out=dApow[:, :, 1:])
    K = np.einsum("dn,dnk->dk", w, dApow)                     # (D,L)
    K[:, 0] += Dvec.astype(np.float64)                        # fold skip

    # Toeplitz lhsT: T[d][j,i] = K[d, i-j] for i>=j
    T = np.zeros((D_MODEL, L, L), np.float32)
    Kf = K.astype(np.float32)
    for k in range(L):
        idx = np.arange(L - k)
        T[:, idx, idx + k] = Kf[:, k][:, None]

    # E lhsT [d, j, n] = dA^(L-1-j) * dB
    E = (dApow[:, :, ::-1] * dB[:, :, None]).transpose(0, 2, 1)  # (D,L,N)
    # Cb lhsT [d, n, i] = C * dA^(i+1)
    dApow1 = dApow * dA[:, :, None]
    Cb = C.astype(np.float64)[:, :, None] * dApow1               # (D,N,L)
    P = dApow1[:, :, L - 1]                                      # dA^L (D,N)
    return (T.astype(np.float16), E.astype(np.float16),
            Cb.astype(np.float16), P)


def _emit_kernel(nc, tile, mybir, make_identity):
    f16 = mybir.dt.float16
    f32 = mybir.dt.float32
    mult = mybir.AluOpType.mult
    add = mybir.AluOpType.add

    # u pre-gathered on host: [j, dl, t] with t = b*NCH + c
    u_in = nc.dram_tensor("u_in", [L, DPC, BC], f16, kind="ExternalInput").ap()
    # [group, j, ch_in_group, i]
    t_all = nc.dram_tensor("t_all", [DPC // GT, L, GT, L], f16,
                           kind="ExternalInput").ap()
    e_all = nc.dram_tensor("e_all", [DPC // GE, L, GE, N_STATE], f16,
                           kind="ExternalInput").ap()
    # [group, p=(par,n), ch_in_group, i] — Cb zero-padded to K=128
    cb_all = nc.dram_tensor("cb_all", [DPC // GQ, 128, GQ, L], f16,
                            kind="ExternalInput").ap()
    # [p=(par,n), q, c]: dA^L, zeroed at c=0 (scan-chain reset)
    pmul = nc.dram_tensor("pmul", [128, 64, NCH], f16,
                          kind="ExternalInput").ap()
    w2 = nc.dram_tensor("w2", [DPC, VOCAB], f16, kind="ExternalInput").ap()
    out_t = nc.dram_tensor("out_t", [VOCAB, BC * L], f16,
                           kind="ExternalOutput").ap()

    with tile.TileContext(nc) as tc, ExitStack() as ctx:
        persist = ctx.enter_context(tc.tile_pool(name="persist", bufs=1))
        u_sb = persist.tile([128, DPC, BC], f16, name="u_sb")    # [j, dl, t]
        y_sb = persist.tile([128, DPC, BC], f16, name="y_sb")    # [i, dl, t]
        # xin: [p=(par,n), b, q, c]; slot c holds S[c-1] (c=0 is zero);
        # tensor_tensor_scan rewrites slot c with hstart[c] in place.
        xin = persist.tile([128, BATCH, 64, NCH], f16, name="xin")
        pm_sb = persist.tile([128, 64, NCH], f16, name="pm_sb")
        w2_sb = persist.tile([DPC, VOCAB], f16, name="w2_sb")
        ident = persist.tile([128, 128], f16, name="ident")

        make_identity(nc, ident)
        nc.gpsimd.dma_start(out=w2_sb, in_=w2)
        nc.gpsimd.dma_start(out=pm_sb, in_=pmul)
        nc.vector.memset(xin[:, :, :, 0], 0.0)

        # ---- Phase A: load u (host-gathered embedding) on gpsimd while
        # the E/T/Cb operators stream in on the sync/scalar HWDGE queues ----
        for g in range(DPC // GU):
            nc.gpsimd.dma_start(out=u_sb[:, g * GU:(g + 1) * GU, :],
                                in_=u_in[:, g * GU:(g + 1) * GU, :])

        cp_engines = [nc.vector, nc.scalar]

        def cp(k, out, in_):
            e = cp_engines[k % 2]
            if e is nc.scalar:
                e.copy(out, in_)
            else:
                e.tensor_copy(out, in_)

        # ---- Phase B: chunk-end states S[c] -> xin slots 1..31 ----
        # ---- Phase C: hardware scan (fp32 state) -> hstart in place ----
        def emit_scans(half):
            sl = slice(32 * half, 32 * (half + 1))
            for b in range(BATCH):
                ap = xin[:, b, sl, :].opt()
                nc.vector.tensor_tensor_scan(out=ap, data0=pm_sb[:, sl, :].opt(),
                                             data1=ap, initial=0.0,
                                             op0=mult, op1=add)

        with tc.tile_pool(name="ewp", bufs=DPC // GE) as ewp, \
             tc.tile_pool(name="ps_s", bufs=4, space="PSUM") as ps_s:
            e_ws = []
            for g in range(DPC // GE):
                e_w = ewp.tile([L, GE, N_STATE], f16, name="e_w")
                nc.sync.dma_start(out=e_w, in_=e_all[g])
                e_ws.append(e_w)
            for q in range(64):
                e_w = e_ws[q // (GE // 2)]
                s_ps = ps_s.tile([128, BATCH, NCH], f32, name="s_ps")
                for par in range(2):
                    nc.tensor.matmul(
                        s_ps[64 * par:64 * (par + 1), :, :],
                        lhsT=e_w[:, 2 * (q % (GE // 2)) + par, :],
                        rhs=u_sb[:, 2 * q + par, :],
                        start=True, stop=True,
                        tile_position=(0, 64 * par) if par else None)
                # S[0..30] -> xin[:, :, q, 1:32]; both iterate (b, c)
                nc.scalar.copy(xin[:, :, q, 1:], s_ps[:, :, 0:NCH - 1])
                if q == 31:
                    emit_scans(0)
            emit_scans(1)

        # ---- Phase D: y = T^T u (+) Cb^T hstart -> y_sb ----
        with tc.tile_pool(name="twp", bufs=3) as twp, \
             tc.tile_pool(name="cbp", bufs=2) as cbp, \
             tc.tile_pool(name="ps_y", bufs=AHEAD, space="PSUM") as ps_y:
            t_w = [None]
            cb_w = [None]
            tps = {}

            def emit_T(q):
                if q % (GT // 2) == 0:
                    t_w[0] = twp.tile([L, GT, L], f16, name="t_w")
                    nc.scalar.dma_start(out=t_w[0], in_=t_all[q // (GT // 2)])
                y_ps = ps_y.tile([L, 2, BC], f32, name="y_ps")
                for par in range(2):
                    dl = 2 * q + par
                    nc.tensor.matmul(y_ps[:, par, :], lhsT=t_w[0][:, dl % GT, :],
                                     rhs=u_sb[:, dl, :], start=(par == 0),
                                     stop=False)
                tps[q] = y_ps

            def emit_cb(q):
                if (2 * q) % GQ == 0:
                    cb_w[0] = cbp.tile([128, GQ, L], f16, name="cb_w")
                    nc.sync.dma_start(out=cb_w[0], in_=cb_all[(2 * q) // GQ])
                h = xin[:, :, q, :]
                for par in range(2):
                    dl = 2 * q + par
                    nc.tensor.matmul(
                        tps[q][:, par, :],
                        lhsT=cb_w[0][:, dl % GQ, :],
                        rhs=h, start=False, stop=(par == 1))

            for q in range(AHEAD):
                emit_T(q)
            for q in range(64):
                emit_cb(q)
                cp(q, y_sb[:, 2 * q:2 * q + 2, :], tps[q])
                del tps[q]
                if q + AHEAD < 64:
                    emit_T(q + AHEAD)

        # ---- Phase E: PE-transpose y tiles + output projection ----
        # 8 tiles per group: 8 transposes -> one copy; 4 proj pairs -> one copy
        with tc.tile_pool(name="ytp", bufs=3) as ytp, \
             tc.tile_pool(name="lop", bufs=3) as lop, \
             tc.tile_pool(name="ps_t", bufs=2, space="PSUM") as ps_t, \
             tc.tile_pool(name="ps_o", bufs=2, space="PSUM") as ps_o:
            for g in range(BC // 8):
                tp = ps_t.tile([128, 8, L], f16, name="tp")
                for i in range(8):
                    nc.tensor.transpose(tp[:, i, :], y_sb[:, :, 8 * g + i],
                                        ident)
                yt = ytp.tile([128, 8, L], f16, name="yt")
                cp(g, yt, tp)
                lo_ps = ps_o.tile([128, 2, 2 * L], f32, name="lo_ps")
                for p in range(4):
                    nc.tensor.matmul(
                        lo_ps[64 * (p % 2):64 * (p % 2) + 64, p // 2, :],
                        lhsT=w2_sb, rhs=yt[:, 2 * p:2 * p + 2, :],
                        start=True, stop=True,
                        tile_position=(0, 64) if p % 2 else None)
                lo = lop.tile([128, 2, 2 * L], f16, name="lo")
                cp(g + 1, lo, lo_ps)
                base = g * 8 * L
                oap = out_t[:, base:base + 8 * L].rearrange(
                    "v (s x) -> v s x", s=2)
                nc.gpsimd.dma_start(out=oap[:, :, 0:2 * L], in_=lo[0:64])
                nc.gpsimd.dma_start(out=oap[:, :, 2 * L:4 * L], in_=lo[64:128])


def _build_nc():
    import concourse.tile as tile
    from concourse import bacc, mybir
    from concourse.masks import make_identity

    nc = bacc.Bacc(trn_type="TRN2", target_bir_lowering=False, debug=False)
    _emit_kernel(nc, tile, mybir, make_identity)
    nc.compile()
    return nc


_NC_CACHE = None


def kernel(x, emb, log_neg_A, B, C, D, log_dt, W_out, b_out):
    global LAST_RESULTS, _NC_CACHE
    from concourse.bass_utils import run_bass_kernel_spmd

    x = np.asarray(x).astype(np.int64)
    emb = np.asarray(emb, np.float32)
    log_neg_A = np.asarray(log_neg_A, np.float32)
    B_in = np.asarray(B, np.float32)
    C = np.asarray(C, np.float32)
    D_in = np.asarray(D, np.float32)
    log_dt = np.asarray(log_dt, np.float32)
    W_out = np.asarray(W_out, np.float32)
    b_out = np.asarray(b_out, np.float32)

    T, E, Cb, P = _precompute_host(emb, log_neg_A, B_in, C, D_in, log_dt, W_out)

    # u = emb[x] in fp16; tile order t = b*NCH + c
    emb16 = emb.astype(np.float16)
    u_full = emb16[x].reshape(BATCH, NCH, L, D_MODEL)

    in_maps = []
    for core in range(NCORES):
        ds = slice(core * DPC, (core + 1) * DPC)
        # u_in: [j, dl, t=(b, c)]
        uc = np.ascontiguousarray(
            u_full[:, :, :, ds].transpose(2, 3, 0, 1).reshape(L, DPC, BC))
        # pmul layout [p=(par,n), q, c]: p = 64*par + n, d = 2*q + par
        Pc = P[ds].reshape(64, 2, N_STATE).transpose(1, 2, 0).reshape(128, 64)
        pm = np.broadcast_to(Pc[:, :, None], (128, 64, NCH)).copy()
        pm[:, :, 0] = 0.0
        # t_all: [DPC,L,L] -> [DPC/GT, L, GT, L] (ch groups, j-major)
        Tc = np.ascontiguousarray(
            T[ds].reshape(DPC // GT, GT, L, L).transpose(0, 2, 1, 3))
        # e_all: [DPC,L,N] -> [DPC/GE, L, GE, N]
        Ec = np.ascontiguousarray(
            E[ds].reshape(DPC // GE, GE, L, N_STATE).transpose(0, 2, 1, 3))
        # cb_all: [DPC,N,L] zero-padded to [DPC,128,L]: rows 64*(d%2)+n
        Cbpad = np.zeros((DPC, 128, L), np.float16)
        for par in range(2):
            Cbpad[par::2, 64 * par:64 * (par + 1), :] = Cb[ds][par::2]
        Cbc = np.ascontiguousarray(
            Cbpad.reshape(DPC // GQ, GQ, 128, L).transpose(0, 2, 1, 3))
        in_maps.append({
            "u_in": uc,
            "t_all": Tc,
            "e_all": Ec,
            "cb_all": Cbc,
            "pmul": pm.astype(np.float16),
            "w2": np.ascontiguousarray(W_out[ds]).astype(np.float16),
        })

    if _NC_CACHE is None:
        _NC_CACHE = _build_nc()
    nc = _NC_CACHE

    trace = bool(int(os.environ.get("BASS_TRACE", "0") or "0"))
    LAST_RESULTS = run_bass_kernel_spmd(
        nc, in_maps, core_ids=list(range(NCORES)), trace=trace)

    logitsT = np.zeros((VOCAB, BC * L), np.float64)
    for r in LAST_RESULTS.results:
        logitsT += r["out_t"].astype(np.float64)
    # columns tok = (b*NCH + c)*L + i
    out = logitsT.T.reshape(BATCH, NCH, L, VOCAB).reshape(BATCH, SEQ, VOCAB)
    return (out + b_out.astype(np.float64)).astype(np.float32)


# revision 15
# speedup vs baseline: 1.9696x; 1.0183x over previous
"""S4D "CopyingModel" Trainium2 Bass kernel (v4 — contiguous-copy pipeline).

Math: logits = (s4d_scan(emb[x]) + emb[x]*D) @ W_out + b_out, with a
per-channel diagonal SSM (d_model=1024 channels, d_state=64).

Strategy (8 NeuronCores, channel-sharded: 128 channels per core, every core
sees all 8 batches x 4096 tokens):
  - host precomputes (f64) the discretized per-channel operators and ships
    them as fp16 matmul weights:
      * T[d]  [L,L]  lower-tri Toeplitz of the truncated conv kernel
               K[d,k] = sum_n C*dB*dA^k (skip D folded into K[d,0])
      * E[d]  [L,N]  chunk-end state accumulator dA^(L-1-j)*dB
      * Cb[d] [N,L]  state->output C*dA^(i+1), zero-padded to K=128 rows
      * pmul  [p,q,c] chunk-to-chunk decay dA^L (0 at c=0 to reset chains)
  - u = emb[x] gathered on host, shipped per-core as [j, dl, t] fp16 so
    every matmul rhs read and every engine copy is contiguous
  - sequence chunked: L=128, tiles t = b*32 + c (batch-major)
  - chunk-carry handled by hardware tensor_tensor_scan (fp32 state) on DVE,
    overlapped with the Toeplitz matmuls
  - output projection via PE transposes + fp16 logits^T partials;
    host sums cores + bias.
"""

import os
from contextlib import ExitStack

import numpy as np

BATCH = 8
SEQ = 4096
D_MODEL = 1024
N_STATE = 64
VOCAB = 64
L = 128                   # chunk length
NCH = SEQ // L            # 32 chunks
NCORES = 8
DPC = D_MODEL // NCORES   # 128 channels per core
BC = NCH * BATCH          # 256 (batch, chunk) tiles; index t = b*NCH + c

# DMA batch sizes
GU = 16   # u channel-columns per DMA
GT = 16   # T (Toeplitz) channels per DMA
GE = 16   # E channels per DMA
GQ = 8    # Cb channels per DMA
AHEAD = 7 # T-matmul channel-pairs in flight ahead of Cb in phase D

LAST_RESULTS = None       # BassKernelResults of the most recent run (for test.py)


def _precompute_host(emb, log_neg_A, Bmat, C, Dvec, log_dt, W_out):
    """Float64 host precompute of all device operands."""
    dt = np.exp(log_dt.astype(np.float64))                    # (D,)
    A = -np.exp(log_neg_A.astype(np.float64))                 # (D,N)
    dA = np.exp(dt[:, None] * A)                              # (D,N)
    dB = (dA - 1.0) / A * Bmat.astype(np.float64)             # (D,N)
    w = C.astype(np.float64) * dB                             # (D,N)

    # dApow[d,n,k] = dA^k, k=0..L-1
    dApow = np.ones((D_MODEL, N_STATE, L))
    np.cumprod(np.broadcast_to(dA[:, :, None], (D_MODEL, N_STATE, L - 1)),
               axis=2, out=dApow[:, :, 1:])
    K = np.einsum("dn,dnk->dk", w, dApow)                     # (D,L)
    K[:, 0] += Dvec.astype(np.float64)                        # fold skip

    # Toeplitz lhsT: T[d][j,i] = K[d, i-j] for i>=j
    T = np.zeros((D_MODEL, L, L), np.float32)
    Kf = K.astype(np.float32)
    for k in range(L):
        idx = np.arange(L - k)
        T[:, idx, idx + k] = Kf[:, k][:, None]

    # E lhsT [d, j, n] = dA^(L-1-j) * dB
    E = (dApow[:, :, ::-1] * dB[:, :, None]).transpose(0, 2, 1)  # (D,L,N)
    # Cb lhsT [d, n, i] = C * dA^(i+1)
    dApow1 = dApow * dA[:, :, None]
    Cb = C.astype(np.float64)[:, :, None] * dApow1               # (D,N,L)
    P = dApow1[:, :, L - 1]                                      # dA^L (D,N)
    return (T.astype(np.float16), E.astype(np.float16),
            Cb.astype(np.float16), P)


def _emit_kernel(nc, tile, mybir, make_identity):
    f16 = mybir.dt.float16
    f32 = mybir.dt.float32
    mult = mybir.AluOpType.mult
    add = mybir.AluOpType.add

    # u pre-gathered on host: [j, dl, t] with t = b*NCH + c
    u_in = nc.dram_tensor("u_in", [L, DPC, BC], f16, kind="ExternalInput").ap()
    # [group, j, ch_in_group, i]
    t_all = nc.dram_tensor("t_all", [DPC // GT, L, GT, L], f16,
                           kind="ExternalInput").ap()
    e_all = nc.dram_tensor("e_all", [DPC // GE, L, GE, N_STATE], f16,
                           kind="ExternalInput").ap()
    # [group, p=(par,n), ch_in_group, i] — Cb zero-padded to K=128
    cb_all = nc.dram_tensor("cb_all", [DPC // GQ, 128, GQ, L], f16,
                            kind="ExternalInput").ap()
    # [p=(par,n), q, c]: dA^L, zeroed at c=0 (scan-chain reset)
    pmul = nc.dram_tensor("pmul", [128, 64, NCH], f16,
                          kind="ExternalInput").ap()
    w2 = nc.dram_tensor("w2", [DPC, VOCAB], f16, kind="ExternalInput").ap()
    out_t = nc.dram_tensor("out_t", [VOCAB, BC * L], f16,
                           kind="ExternalOutput").ap()

    with tile.TileContext(nc) as tc, ExitStack() as ctx:
        persist = ctx.enter_context(tc.tile_pool(name="persist", bufs=1))
        u_sb = persist.tile([128, DPC, BC], f16, name="u_sb")    # [j, dl, t]
        y_sb = persist.tile([128, DPC, BC], f16, name="y_sb")    # [i, dl, t]
        # xin: [p=(par,n), b, q, c]; slot c holds S[c-1] (c=0 is zero);
        # tensor_tensor_scan rewrites slot c with hstart[c] in place.
        xin = persist.tile([128, BATCH, 64, NCH], f16, name="xin")
        pm_sb = persist.tile([128, 64, NCH], f16, name="pm_sb")
        w2_sb = persist.tile([DPC, VOCAB], f16, name="w2_sb")
        ident = persist.tile([128, 128], f16, name="ident")

        make_identity(nc, ident)
        nc.gpsimd.dma_start(out=w2_sb, in_=w2)
        nc.gpsimd.dma_start(out=pm_sb, in_=pmul)
        nc.vector.memset(xin[:, :, :, 0], 0.0)

        # ---- Phase A: load u (host-gathered embedding) on gpsimd while
        # the E/T/Cb operators stream in on the sync/scalar HWDGE queues ----
        for g in range(DPC // GU):
            nc.gpsimd.dma_start(out=u_sb[:, g * GU:(g + 1) * GU, :],
                                in_=u_in[:, g * GU:(g + 1) * GU, :])

        cp_engines = [nc.vector, nc.scalar]

        def cp(k, out, in_):
            e = cp_engines[k % 2]
            if e is nc.scalar:
                e.copy(out, in_)
            else:
                e.tensor_copy(out, in_)

        # ---- Phase B: chunk-end states S[c] -> xin slots 1..31 ----
        # ---- Phase C: hardware scan (fp32 state) -> hstart in place ----
        def emit_scans(half):
            sl = slice(32 * half, 32 * (half + 1))
            for b in range(BATCH):
                ap = xin[:, b, sl, :].opt()
                nc.vector.tensor_tensor_scan(out=ap, data0=pm_sb[:, sl, :].opt(),
                                             data1=ap, initial=0.0,
                                             op0=mult, op1=add)

        with tc.tile_pool(name="ewp", bufs=DPC // GE) as ewp, \
             tc.tile_pool(name="ps_s", bufs=4, space="PSUM") as ps_s:
            e_ws = []
            for g in range(DPC // GE):
                e_w = ewp.tile([L, GE, N_STATE], f16, name="e_w")
                nc.sync.dma_start(out=e_w, in_=e_all[g])
                e_ws.append(e_w)
            for q in range(64):
                e_w = e_ws[q // (GE // 2)]
                s_ps = ps_s.tile([128, BATCH, NCH], f32, name="s_ps")
                for par in range(2):
                    nc.tensor.matmul(
                        s_ps[64 * par:64 * (par + 1), :, :],
                        lhsT=e_w[:, 2 * (q % (GE // 2)) + par, :],
                        rhs=u_sb[:, 2 * q + par, :],
                        start=True, stop=True,
                        tile_position=(0, 64 * par) if par else None)
                # S[0..30] -> xin[:, :, q, 1:32]; both iterate (b, c)
                nc.scalar.copy(xin[:, :, q, 1:], s_ps[:, :, 0:NCH - 1])
                if q == 31:
                    emit_scans(0)
            emit_scans(1)

        # ---- Phase D: y = T^T u (+) Cb^T hstart -> y_sb ----
        with tc.tile_pool(name="twp", bufs=3) as twp, \
             tc.tile_pool(name="cbp", bufs=2) as cbp, \
             tc.tile_pool(name="ps_y", bufs=AHEAD, space="PSUM") as ps_y:
            t_w = [None]
            cb_w = [None]
            tps = {}

            def emit_T(q):
                if q % (GT // 2) == 0:
                    t_w[0] = twp.tile([L, GT, L], f16, name="t_w")
                    nc.scalar.dma_start(out=t_w[0], in_=t_all[q // (GT // 2)])
                y_ps = ps_y.tile([L, 2, BC], f32, name="y_ps")
                for par in range(2):
                    dl = 2 * q + par
                    nc.tensor.matmul(y_ps[:, par, :], lhsT=t_w[0][:, dl % GT, :],
                                     rhs=u_sb[:, dl, :], start=(par == 0),
                                     stop=False)
                tps[q] = y_ps

            def emit_cb(q):
                if (2 * q) % GQ == 0:
                    cb_w[0] = cbp.tile([128, GQ, L], f16, name="cb_w")
                    nc.sync.dma_start(out=cb_w[0], in_=cb_all[(2 * q) // GQ])
                h = xin[:, :, q, :]
                for par in range(2):
                    dl = 2 * q + par
                    nc.tensor.matmul(
                        tps[q][:, par, :],
                        lhsT=cb_w[0][:, dl % GQ, :],
                        rhs=h, start=False, stop=(par == 1))

            for q in range(AHEAD):
                emit_T(q)
            for q in range(64):
                emit_cb(q)
                cp(q, y_sb[:, 2 * q:2 * q + 2, :], tps[q])
                del tps[q]
                if q + AHEAD < 64:
                    emit_T(q + AHEAD)

        # ---- Phase E: PE-transpose y tiles + output projection ----
        # 8 tiles per group: 8 transposes -> one copy; 4 proj pairs -> one copy
        with tc.tile_pool(name="ytp", bufs=3) as ytp, \
             tc.tile_pool(name="lop", bufs=3) as lop, \
             tc.tile_pool(name="ps_t", bufs=3, space="PSUM") as ps_t, \
             tc.tile_pool(name="ps_o", bufs=2, space="PSUM") as ps_o:
            for g in range(BC // 8):
                tp = ps_t.tile([128, 8, L], f16, name="tp")
                for i in range(8):
                    nc.tensor.transpose(tp[:, i, :], y_sb[:, :, 8 * g + i],
                                        ident)
                yt = ytp.tile([128, 8, L], f16, name="yt")
                nc.vector.tensor_copy(yt[:, 0:4, :], tp[:, 0:4, :])
                nc.scalar.copy(yt[:, 4:8, :], tp[:, 4:8, :])
                lo_ps = ps_o.tile([128, 2, 2 * L], f32, name="lo_ps")
                for p in range(4):
                    nc.tensor.matmul(
                        lo_ps[64 * (p % 2):64 * (p % 2) + 64, p // 2, :],
                        lhsT=w2_sb, rhs=yt[:, 2 * p:2 * p + 2, :],
                        start=True, stop=True,
                        tile_position=(0, 64) if p % 2 else None)
                lo = lop.tile([128, 2, 2 * L], f16, name="lo")
                nc.vector.tensor_copy(lo[:, 0, :], lo_ps[:, 0, :])
                nc.scalar.copy(lo[:, 1, :], lo_ps[:, 1, :])
                base = g * 8 * L
                oap = out_t[:, base:base + 8 * L].rearrange(
                    "v (s x) -> v s x", s=2)
                eng = (nc.gpsimd, nc.sync, nc.scalar)[g % 3]
                eng.dma_start(out=oap[:, :, 0:2 * L], in_=lo[0:64])
                eng.dma_start(out=oap[:, :, 2 * L:4 * L], in_=lo[64:128])


def _build_nc():
    import concourse.tile as tile
    from concourse import bacc, mybir
    from concourse.masks import make_identity

    nc = bacc.Bacc(trn_type="TRN2", target_bir_lowering=False, debug=False)
    _emit_kernel(nc, tile, mybir, make_identity)
    nc.compile()
    return nc


_NC_CACHE = None


def kernel(x, emb, log_neg_A, B, C, D, log_dt, W_out, b_out):
    global LAST_RESULTS, _NC_CACHE
    from concourse.bass_utils import run_bass_kernel_spmd

    x = np.asarray(x).astype(np.int64)
    emb = np.asarray(emb, np.float32)
    log_neg_A = np.asarray(log_neg_A, np.float32)
    B_in = np.asarray(B, np.float32)
    C = np.asarray(C, np.float32)
    D_in = np.asarray(D, np.float32)
    log_dt = np.asarray(log_dt, np.float32)
    W_out = np.asarray(W_out, np.float32)
    b_out = np.asarray(b_out, np.float32)

    T, E, Cb, P = _precompute_host(emb, log_neg_A, B_in, C, D_in, log_dt, W_out)

    # u = emb[x] in fp16; tile order t = b*NCH + c
    emb16 = emb.astype(np.float16)
    u_full = emb16[x].reshape(BATCH, NCH, L, D_MODEL)

    in_maps = []
    for core in range(NCORES):
        ds = slice(core * DPC, (core + 1) * DPC)
        # u_in: [j, dl, t=(b, c)]
        uc = np.ascontiguousarray(
            u_full[:, :, :, ds].transpose(2, 3, 0, 1).reshape(L, DPC, BC))
        # pmul layout [p=(par,n), q, c]: p = 64*par + n, d = 2*q + par
        Pc = P[ds].reshape(64, 2, N_STATE).transpose(1, 2, 0).reshape(128, 64)
        pm = np.broadcast_to(Pc[:, :, None], (128, 64, NCH)).copy()
        pm[:, :, 0] = 0.0
        # t_all: [DPC,L,L] -> [DPC/GT, L, GT, L] (ch groups, j-major)
        Tc = np.ascontiguousarray(
            T[ds].reshape(DPC // GT, GT, L, L).transpose(0, 2, 1, 3))
        # e_all: [DPC,L,N] -> [DPC/GE, L, GE, N]
        Ec = np.ascontiguousarray(
            E[ds].reshape(DPC // GE, GE, L, N_STATE).transpose(0, 2, 1, 3))
        # cb_all: [DPC,N,L] zero-padded to [DPC,128,L]: rows 64*(d%2)+n
        Cbpad = np.zeros((DPC, 128, L), np.float16)
        for par in range(2):
            Cbpad[par::2, 64 * par:64 * (par + 1), :] = Cb[ds][par::2]
        Cbc = np.ascontiguousarray(
            Cbpad.reshape(DPC // GQ, GQ, 128, L).transpose(0, 2, 1, 3))
        in_maps.append({
            "u_in": uc,
            "t_all": Tc,
            "e_all": Ec,
            "cb_all": Cbc,
            "pmul": pm.astype(np.float16),
            "w2": np.ascontiguousarray(W_out[ds]).astype(np.float16),
        })

    if _NC_CACHE is None:
        _NC_CACHE = _build_nc()
    nc = _NC_CACHE

    trace = bool(int(os.environ.get("BASS_TRACE", "0") or "0"))
    LAST_RESULTS = run_bass_kernel_spmd(
        nc, in_maps, core_ids=list(range(NCORES)), trace=trace)

    logitsT = np.zeros((VOCAB, BC * L), np.float64)
    for r in LAST_RESULTS.results:
        logitsT += r["out_t"].astype(np.float64)
    # columns tok = (b*NCH + c)*L + i
    out = logitsT.T.reshape(BATCH, NCH, L, VOCAB).reshape(BATCH, SEQ, VOCAB)
    return (out + b_out.astype(np.float64)).astype(np.float32)


# revision 17
# speedup vs baseline: 2.1140x; 1.0733x over previous
"""S4D "CopyingModel" Trainium2 Bass kernel (v4 — contiguous-copy pipeline).

Math: logits = (s4d_scan(emb[x]) + emb[x]*D) @ W_out + b_out, with a
per-channel diagonal SSM (d_model=1024 channels, d_state=64).

Strategy (8 NeuronCores, channel-sharded: 128 channels per core, every core
sees all 8 batches x 4096 tokens):
  - host precomputes (f64) the discretized per-channel operators and ships
    them as fp16 matmul weights:
      * T[d]  [L,L]  lower-tri Toeplitz of the truncated conv kernel
               K[d,k] = sum_n C*dB*dA^k (skip D folded into K[d,0])
      * E[d]  [L,N]  chunk-end state accumulator dA^(L-1-j)*dB
      * Cb[d] [N,L]  state->output C*dA^(i+1), zero-padded to K=128 rows
      * pmul  [p,q,c] chunk-to-chunk decay dA^L (0 at c=0 to reset chains)
  - u = emb[x] gathered on host, shipped per-core as [j, dl, t] fp16 so
    every matmul rhs read and every engine copy is contiguous
  - sequence chunked: L=128, tiles t = b*32 + c (batch-major)
  - chunk-carry handled by hardware tensor_tensor_scan (fp32 state) on DVE,
    overlapped with the Toeplitz matmuls
  - output projection via PE transposes + fp16 logits^T partials;
    host sums cores + bias.
"""

import os
from contextlib import ExitStack

import numpy as np

BATCH = 8
SEQ = 4096
D_MODEL = 1024
N_STATE = 64
VOCAB = 64
L = 128                   # chunk length
NCH = SEQ // L            # 32 chunks
NCORES = 8
DPC = D_MODEL // NCORES   # 128 channels per core
BC = NCH * BATCH          # 256 (batch, chunk) tiles; index t = b*NCH + c

# DMA batch sizes
GU = 8    # u channel-columns per DMA
GT = 16   # T (Toeplitz) channels per DMA
GE = 16   # E channels per DMA
GQ = 8    # Cb channels per DMA
AHEAD = 7 # T-matmul channel-pairs in flight ahead of Cb in phase D

LAST_RESULTS = None       # BassKernelResults of the most recent run (for test.py)


def _precompute_host(emb, log_neg_A, Bmat, C, Dvec, log_dt, W_out):
    """Float64 host precompute of all device operands."""
    dt = np.exp(log_dt.astype(np.float64))                    # (D,)
    A = -np.exp(log_neg_A.astype(np.float64))                 # (D,N)
    dA = np.exp(dt[:, None] * A)                              # (D,N)
    dB = (dA - 1.0) / A * Bmat.astype(np.float64)             # (D,N)
    w = C.astype(np.float64) * dB                             # (D,N)

    # dApow[d,n,k] = dA^k, k=0..L-1
    dApow = np.ones((D_MODEL, N_STATE, L))
    np.cumprod(np.broadcast_to(dA[:, :, None], (D_MODEL, N_STATE, L - 1)),
               axis=2, out=dApow[:, :, 1:])
    K = np.einsum("dn,dnk->dk", w, dApow)                     # (D,L)
    K[:, 0] += Dvec.astype(np.float64)                        # fold skip

    # Toeplitz lhsT: T[d][j,i] = K[d, i-j] for i>=j
    T = np.zeros((D_MODEL, L, L), np.float32)
    Kf = K.astype(np.float32)
    for k in range(L):
        idx = np.arange(L - k)
        T[:, idx, idx + k] = Kf[:, k][:, None]

    # E lhsT [d, j, n] = dA^(L-1-j) * dB
    E = (dApow[:, :, ::-1] * dB[:, :, None]).transpose(0, 2, 1)  # (D,L,N)
    # Cb lhsT [d, n, i] = C * dA^(i+1)
    dApow1 = dApow * dA[:, :, None]
    Cb = C.astype(np.float64)[:, :, None] * dApow1               # (D,N,L)
    P = dApow1[:, :, L - 1]                                      # dA^L (D,N)
    return (T.astype(np.float16), E.astype(np.float16),
            Cb.astype(np.float16), P)


def _emit_kernel(nc, tile, mybir, make_identity):
    f16 = mybir.dt.float16
    f32 = mybir.dt.float32
    mult = mybir.AluOpType.mult
    add = mybir.AluOpType.add

    # u pre-gathered on host: [j, dl, t] with t = b*NCH + c
    u_in = nc.dram_tensor("u_in", [L, DPC, BC], f16, kind="ExternalInput").ap()
    # [group, j, ch_in_group, i]
    t_all = nc.dram_tensor("t_all", [DPC // GT, L, GT, L], f16,
                           kind="ExternalInput").ap()
    e_all = nc.dram_tensor("e_all", [DPC // GE, L, GE, N_STATE], f16,
                           kind="ExternalInput").ap()
    # [group, p=(par,n), ch_in_group, i] — Cb zero-padded to K=128
    cb_all = nc.dram_tensor("cb_all", [DPC // GQ, 128, GQ, L], f16,
                            kind="ExternalInput").ap()
    # [p=(par,n), q, c]: dA^L, zeroed at c=0 (scan-chain reset)
    pmul = nc.dram_tensor("pmul", [128, 64, NCH], f16,
                          kind="ExternalInput").ap()
    w2 = nc.dram_tensor("w2", [DPC, VOCAB], f16, kind="ExternalInput").ap()
    out_t = nc.dram_tensor("out_t", [VOCAB, BC * L], f16,
                           kind="ExternalOutput").ap()

    with tile.TileContext(nc) as tc, ExitStack() as ctx:
        persist = ctx.enter_context(tc.tile_pool(name="persist", bufs=1))
        u_sb = persist.tile([128, DPC, BC], f16, name="u_sb")    # [j, dl, t]
        y_sb = persist.tile([128, DPC, BC], f16, name="y_sb")    # [i, dl, t]
        # xin: [p=(par,n), b, q, c]; slot c holds S[c-1] (c=0 is zero);
        # tensor_tensor_scan rewrites slot c with hstart[c] in place.
        xin = persist.tile([128, BATCH, 64, NCH], f16, name="xin")
        pm_sb = persist.tile([128, 64, NCH], f16, name="pm_sb")
        w2_sb = persist.tile([DPC, VOCAB], f16, name="w2_sb")
        ident = persist.tile([128, 128], f16, name="ident")

        make_identity(nc, ident)
        nc.sync.dma_start(out=w2_sb, in_=w2)
        nc.sync.dma_start(out=pm_sb, in_=pmul)
        nc.vector.memset(xin[:, :, :, 0], 0.0)

        # ---- Phase A: load u (host-gathered embedding) on gpsimd while
        # the E/T/Cb operators stream in on the sync/scalar HWDGE queues ----
        for g in range(DPC // GU):
            nc.gpsimd.dma_start(out=u_sb[:, g * GU:(g + 1) * GU, :],
                                in_=u_in[:, g * GU:(g + 1) * GU, :])

        cp_engines = [nc.vector, nc.scalar]

        def cp(k, out, in_):
            e = cp_engines[k % 2]
            if e is nc.scalar:
                e.copy(out, in_)
            else:
                e.tensor_copy(out, in_)

        # ---- Phase B: chunk-end states S[c] -> xin slots 1..31 ----
        # ---- Phase C: hardware scan (fp32 state) -> hstart in place ----
        def emit_scans(half):
            sl = slice(32 * half, 32 * (half + 1))
            for b in range(BATCH):
                ap = xin[:, b, sl, :].opt()
                nc.vector.tensor_tensor_scan(out=ap, data0=pm_sb[:, sl, :].opt(),
                                             data1=ap, initial=0.0,
                                             op0=mult, op1=add)

        with tc.tile_pool(name="ewp", bufs=DPC // GE) as ewp, \
             tc.tile_pool(name="ps_s", bufs=4, space="PSUM") as ps_s:
            e_ws = []
            for g in range(DPC // GE):
                e_w = ewp.tile([L, GE, N_STATE], f16, name="e_w")
                nc.sync.dma_start(out=e_w, in_=e_all[g])
                e_ws.append(e_w)
            for q in range(64):
                e_w = e_ws[q // (GE // 2)]
                s_ps = ps_s.tile([128, BATCH, NCH], f32, name="s_ps")
                for par in range(2):
                    nc.tensor.matmul(
                        s_ps[64 * par:64 * (par + 1), :, :],
                        lhsT=e_w[:, 2 * (q % (GE // 2)) + par, :],
                        rhs=u_sb[:, 2 * q + par, :],
                        start=True, stop=True,
                        tile_position=(0, 64 * par) if par else None)
                # S[0..30] -> xin[:, :, q, 1:32]; both iterate (b, c)
                nc.scalar.copy(xin[:, :, q, 1:], s_ps[:, :, 0:NCH - 1])
                if q == 31:
                    emit_scans(0)
            emit_scans(1)

        # ---- Phase D: y = T^T u (+) Cb^T hstart -> y_sb ----
        with tc.tile_pool(name="twp", bufs=3) as twp, \
             tc.tile_pool(name="cbp", bufs=2) as cbp, \
             tc.tile_pool(name="ps_y", bufs=AHEAD, space="PSUM") as ps_y:
            t_w = [None]
            cb_w = [None]
            tps = {}

            def emit_T(q):
                if q % (GT // 2) == 0:
                    t_w[0] = twp.tile([L, GT, L], f16, name="t_w")
                    nc.scalar.dma_start(out=t_w[0], in_=t_all[q // (GT // 2)])
                y_ps = ps_y.tile([L, 2, BC], f32, name="y_ps")
                for par in range(2):
                    dl = 2 * q + par
                    nc.tensor.matmul(y_ps[:, par, :], lhsT=t_w[0][:, dl % GT, :],
                                     rhs=u_sb[:, dl, :], start=(par == 0),
                                     stop=False)
                tps[q] = y_ps

            def emit_cb(q):
                if (2 * q) % GQ == 0:
                    cb_w[0] = cbp.tile([128, GQ, L], f16, name="cb_w")
                    nc.sync.dma_start(out=cb_w[0], in_=cb_all[(2 * q) // GQ])
                h = xin[:, :, q, :]
                for par in range(2):
                    dl = 2 * q + par
                    nc.tensor.matmul(
                        tps[q][:, par, :],
                        lhsT=cb_w[0][:, dl % GQ, :],
                        rhs=h, start=False, stop=(par == 1))

            for q in range(AHEAD):
                emit_T(q)
            for q in range(64):
                emit_cb(q)
                if q < 24:
                    nc.scalar.copy(y_sb[:, 2 * q:2 * q + 2, :], tps[q])
                else:
                    cp(q, y_sb[:, 2 * q:2 * q + 2, :], tps[q])
                del tps[q]
                if q + AHEAD < 64:
                    emit_T(q + AHEAD)

        # ---- Phase E: PE-transpose y tiles + output projection ----
        with tc.tile_pool(name="ytp", bufs=3) as ytp, \
             tc.tile_pool(name="lop", bufs=3) as lop, \
             tc.tile_pool(name="ps_t", bufs=3, space="PSUM") as ps_t, \
             tc.tile_pool(name="ps_o", bufs=2, space="PSUM") as ps_o:
            for g in range(BC // 8):
                tp = ps_t.tile([128, 8, L], f16, name="tp")
                for i in range(8):
                    nc.tensor.transpose(tp[:, i, :], y_sb[:, :, 8 * g + i],
                                        ident)
                yt = ytp.tile([128, 8, L], f16, name="yt")
                nc.vector.tensor_copy(yt[:, 0:4, :], tp[:, 0:4, :])
                nc.scalar.copy(yt[:, 4:8, :], tp[:, 4:8, :])
                lo_ps = ps_o.tile([128, 2, 2 * L], f32, name="lo_ps")
                for p in range(4):
                    nc.tensor.matmul(
                        lo_ps[64 * (p % 2):64 * (p % 2) + 64, p // 2, :],
                        lhsT=w2_sb, rhs=yt[:, 2 * p:2 * p + 2, :],
                        start=True, stop=True,
                        tile_position=(0, 64) if p % 2 else None)
                lo = lop.tile([128, 2, 2 * L], f16, name="lo")
                nc.vector.tensor_copy(lo[:, 0, :], lo_ps[:, 0, :])
                nc.scalar.copy(lo[:, 1, :], lo_ps[:, 1, :])
                base = g * 8 * L
                oap = out_t[:, base:base + 8 * L].rearrange(
                    "v (s x) -> v s x", s=2)
                nc.gpsimd.dma_start(out=oap[:, :, 0:2 * L], in_=lo[0:64])
                nc.gpsimd.dma_start(out=oap[:, :, 2 * L:4 * L], in_=lo[64:128])


def _build_nc():
    import concourse.tile as tile
    from concourse import bacc, mybir
    from concourse.masks import make_identity

    nc = bacc.Bacc(trn_type="TRN2", target_bir_lowering=False, debug=False)
    _emit_kernel(nc, tile, mybir, make_identity)
    nc.compile()
    return nc


_NC_CACHE = None


def kernel(x, emb, log_neg_A, B, C, D, log_dt, W_out, b_out):
    global LAST_RESULTS, _NC_CACHE
    from concourse.bass_utils import run_bass_kernel_spmd

    x = np.asarray(x).astype(np.int64)
    emb = np.asarray(emb, np.float32)
    log_neg_A = np.asarray(log_neg_A, np.float32)
    B_in = np.asarray(B, np.float32)
    C = np.asarray(C, np.float32)
    D_in = np.asarray(D, np.float32)
    log_dt = np.asarray(log_dt, np.float32)
    W_out = np.asarray(W_out, np.float32)
    b_out = np.asarray(b_out, np.float32)

    T, E, Cb, P = _precompute_host(emb, log_neg_A, B_in, C, D_in, log_dt, W_out)

    # u = emb[x] in fp16; tile order t = b*NCH + c
    emb16 = emb.astype(np.float16)
    u_full = emb16[x].reshape(BATCH, NCH, L, D_MODEL)

    in_maps = []
    for core in range(NCORES):
        ds = slice(core * DPC, (core + 1) * DPC)
        # u_in: [j, dl, t=(b, c)]
        uc = np.ascontiguousarray(
            u_full[:, :, :, ds].transpose(2, 3, 0, 1).reshape(L, DPC, BC))
        # pmul layout [p=(par,n), q, c]: p = 64*par + n, d = 2*q + par
        Pc = P[ds].reshape(64, 2, N_STATE).transpose(1, 2, 0).reshape(128, 64)
        pm = np.broadcast_to(Pc[:, :, None], (128, 64, NCH)).copy()
        pm[:, :, 0] = 0.0
        # t_all: [DPC,L,L] -> [DPC/GT, L, GT, L] (ch groups, j-major)
        Tc = np.ascontiguousarray(
            T[ds].reshape(DPC // GT, GT, L, L).transpose(0, 2, 1, 3))
        # e_all: [DPC,L,N] -> [DPC/GE, L, GE, N]
        Ec = np.ascontiguousarray(
            E[ds].reshape(DPC // GE, GE, L, N_STATE).transpose(0, 2, 1, 3))
        # cb_all: [DPC,N,L] zero-padded to [DPC,128,L]: rows 64*(d%2)+n
        Cbpad = np.zeros((DPC, 128, L), np.float16)
        for par in range(2):
            Cbpad[par::2, 64 * par:64 * (par + 1), :] = Cb[ds][par::2]
        Cbc = np.ascontiguousarray(
            Cbpad.reshape(DPC // GQ, GQ, 128, L).transpose(0, 2, 1, 3))
        in_maps.append({
            "u_in": uc,
            "t_all": Tc,
            "e_all": Ec,
            "cb_all": Cbc,
            "pmul": pm.astype(np.float16),
            "w2": np.ascontiguousarray(W_out[ds]).astype(np.float16),
        })

    if _NC_CACHE is None:
        _NC_CACHE = _build_nc()
    nc = _NC_CACHE

    trace = bool(int(os.environ.get("BASS_TRACE", "0") or "0"))
    LAST_RESULTS = run_bass_kernel_spmd(
        nc, in_maps, core_ids=list(range(NCORES)), trace=trace)

    logitsT = np.zeros((VOCAB, BC * L), np.float64)
    for r in LAST_RESULTS.results:
        logitsT += r["out_t"].astype(np.float64)
    # columns tok = (b*NCH + c)*L + i
    out = logitsT.T.reshape(BATCH, NCH, L, VOCAB).reshape(BATCH, SEQ, VOCAB)
    return (out + b_out.astype(np.float64)).astype(np.float32)


# revision 18
# speedup vs baseline: 2.3240x; 1.0993x over previous
"""S4D "CopyingModel" Trainium2 Bass kernel (v4 — contiguous-copy pipeline).

Math: logits = (s4d_scan(emb[x]) + emb[x]*D) @ W_out + b_out, with a
per-channel diagonal SSM (d_model=1024 channels, d_state=64).

Strategy (8 NeuronCores, channel-sharded: 128 channels per core, every core
sees all 8 batches x 4096 tokens):
  - host precomputes (f64) the discretized per-channel operators and ships
    them as fp16 matmul weights:
      * T[d]  [L,L]  lower-tri Toeplitz of the truncated conv kernel
               K[d,k] = sum_n C*dB*dA^k (skip D folded into K[d,0])
      * E[d]  [L,N]  chunk-end state accumulator dA^(L-1-j)*dB
      * Cb[d] [N,L]  state->output C*dA^(i+1), zero-padded to K=128 rows
      * pmul  [p,q,c] chunk-to-chunk decay dA^L (0 at c=0 to reset chains)
  - u = emb[x] gathered on host, shipped per-core as [j, dl, t] fp16 so
    every matmul rhs read and every engine copy is contiguous
  - sequence chunked: L=128, tiles t = b*32 + c (batch-major)
  - chunk-carry handled by hardware tensor_tensor_scan (fp32 state) on DVE,
    overlapped with the Toeplitz matmuls
  - output projection via PE transposes + fp16 logits^T partials;
    host sums cores + bias.
"""

import os
from contextlib import ExitStack

import numpy as np

BATCH = 8
SEQ = 4096
D_MODEL = 1024
N_STATE = 64
VOCAB = 64
L = 128                   # chunk length
NCH = SEQ // L            # 32 chunks
NCORES = 8
DPC = D_MODEL // NCORES   # 128 channels per core
BC = NCH * BATCH          # 256 (batch, chunk) tiles; index t = b*NCH + c

# DMA batch sizes
GU = 8    # u channel-columns per DMA
GT = 16   # T (Toeplitz) channels per DMA
GE = 16   # E channels per DMA
GQ = 8    # Cb channels per DMA
AHEAD = 8 # T-matmul channel-pairs in flight ahead of Cb in phase D

LAST_RESULTS = None       # BassKernelResults of the most recent run (for test.py)


def _precompute_host(emb, log_neg_A, Bmat, C, Dvec, log_dt, W_out):
    """Float64 host precompute of all device operands."""
    dt = np.exp(log_dt.astype(np.float64))                    # (D,)
    A = -np.exp(log_neg_A.astype(np.float64))                 # (D,N)
    dA = np.exp(dt[:, None] * A)                              # (D,N)
    dB = (dA - 1.0) / A * Bmat.astype(np.float64)             # (D,N)
    w = C.astype(np.float64) * dB                             # (D,N)

    # dApow[d,n,k] = dA^k, k=0..L-1
    dApow = np.ones((D_MODEL, N_STATE, L))
    np.cumprod(np.broadcast_to(dA[:, :, None], (D_MODEL, N_STATE, L - 1)),
               axis=2, out=dApow[:, :, 1:])
    K = np.einsum("dn,dnk->dk", w, dApow)                     # (D,L)
    K[:, 0] += Dvec.astype(np.float64)                        # fold skip

    # Toeplitz lhsT: T[d][j,i] = K[d, i-j] for i>=j
    T = np.zeros((D_MODEL, L, L), np.float32)
    Kf = K.astype(np.float32)
    for k in range(L):
        idx = np.arange(L - k)
        T[:, idx, idx + k] = Kf[:, k][:, None]

    # E lhsT [d, j, n] = dA^(L-1-j) * dB
    E = (dApow[:, :, ::-1] * dB[:, :, None]).transpose(0, 2, 1)  # (D,L,N)
    # Cb lhsT [d, n, i] = C * dA^(i+1)
    dApow1 = dApow * dA[:, :, None]
    Cb = C.astype(np.float64)[:, :, None] * dApow1               # (D,N,L)
    P = dApow1[:, :, L - 1]                                      # dA^L (D,N)
    return (T.astype(np.float16), E.astype(np.float16),
            Cb.astype(np.float16), P)


def _emit_kernel(nc, tile, mybir, make_identity):
    f16 = mybir.dt.float16
    f32 = mybir.dt.float32
    mult = mybir.AluOpType.mult
    add = mybir.AluOpType.add

    # u pre-gathered on host: [j, dl, t] with t = b*NCH + c
    u_in = nc.dram_tensor("u_in", [L, DPC, BC], f16, kind="ExternalInput").ap()
    # [group, j, ch_in_group, i]
    t_all = nc.dram_tensor("t_all", [DPC // GT, L, GT, L], f16,
                           kind="ExternalInput").ap()
    e_all = nc.dram_tensor("e_all", [DPC // GE, L, GE, N_STATE], f16,
                           kind="ExternalInput").ap()
    # [group, p=(par,n), ch_in_group, i] — Cb zero-padded to K=128
    cb_all = nc.dram_tensor("cb_all", [DPC // GQ, 128, GQ, L], f16,
                            kind="ExternalInput").ap()
    # [p=(par,n), q, c]: dA^L, zeroed at c=0 (scan-chain reset)
    pmul = nc.dram_tensor("pmul", [128, 64, NCH], f16,
                          kind="ExternalInput").ap()
    w2 = nc.dram_tensor("w2", [DPC, VOCAB], f16, kind="ExternalInput").ap()
    out_t = nc.dram_tensor("out_t", [VOCAB, BC * L], f16,
                           kind="ExternalOutput").ap()

    with tile.TileContext(nc) as tc, ExitStack() as ctx:
        persist = ctx.enter_context(tc.tile_pool(name="persist", bufs=1))
        u_sb = persist.tile([128, DPC, BC], f16, name="u_sb")    # [j, dl, t]
        y_sb = persist.tile([128, DPC, BC], f16, name="y_sb")    # [i, dl, t]
        # xin: [p=(par,n), b, q, c]; slot c holds S[c-1] (c=0 is zero);
        # tensor_tensor_scan rewrites slot c with hstart[c] in place.
        xin = persist.tile([128, BATCH, 64, NCH], f16, name="xin")
        pm_sb = persist.tile([128, 64, NCH], f16, name="pm_sb")
        w2_sb = persist.tile([DPC, VOCAB], f16, name="w2_sb")
        ident = persist.tile([128, 128], f16, name="ident")

        make_identity(nc, ident)
        nc.sync.dma_start(out=w2_sb, in_=w2)
        nc.sync.dma_start(out=pm_sb, in_=pmul)
        nc.vector.memset(xin[:, :, :, 0], 0.0)

        # ---- Phase A: load u (host-gathered embedding) on gpsimd while
        # the E/T/Cb operators stream in on the sync/scalar HWDGE queues ----
        for g in range(DPC // GU):
            nc.gpsimd.dma_start(out=u_sb[:, g * GU:(g + 1) * GU, :],
                                in_=u_in[:, g * GU:(g + 1) * GU, :])

        cp_engines = [nc.vector, nc.scalar]

        def cp(k, out, in_):
            e = cp_engines[k % 2]
            if e is nc.scalar:
                e.copy(out, in_)
            else:
                e.tensor_copy(out, in_)

        # ---- Phase B: chunk-end states S[c] -> xin slots 1..31 ----
        # ---- Phase C: hardware scan (fp32 state) -> hstart in place ----
        def emit_scans(half):
            sl = slice(32 * half, 32 * (half + 1))
            for b in range(BATCH):
                ap = xin[:, b, sl, :].opt()
                nc.vector.tensor_tensor_scan(out=ap, data0=pm_sb[:, sl, :].opt(),
                                             data1=ap, initial=0.0,
                                             op0=mult, op1=add)

        with tc.tile_pool(name="ewp", bufs=DPC // GE) as ewp, \
             tc.tile_pool(name="ps_s", bufs=4, space="PSUM") as ps_s:
            e_ws = []
            for g in range(DPC // GE):
                e_w = ewp.tile([L, GE, N_STATE], f16, name="e_w")
                nc.sync.dma_start(out=e_w, in_=e_all[g])
                e_ws.append(e_w)
            for q in range(64):
                e_w = e_ws[q // (GE // 2)]
                s_ps = ps_s.tile([128, BATCH, NCH], f32, name="s_ps")
                for par in range(2):
                    nc.tensor.matmul(
                        s_ps[64 * par:64 * (par + 1), :, :],
                        lhsT=e_w[:, 2 * (q % (GE // 2)) + par, :],
                        rhs=u_sb[:, 2 * q + par, :],
                        start=True, stop=True,
                        tile_position=(0, 64 * par) if par else None)
                # S[0..30] -> xin[:, :, q, 1:32]; both iterate (b, c)
                nc.scalar.copy(xin[:, :, q, 1:], s_ps[:, :, 0:NCH - 1])
                if q == 31:
                    emit_scans(0)
            emit_scans(1)

        # ---- Phase D: y = T^T u (+) Cb^T hstart -> y_sb ----
        with tc.tile_pool(name="twp", bufs=3) as twp, \
             tc.tile_pool(name="cbp", bufs=2) as cbp, \
             tc.tile_pool(name="ps_y", bufs=AHEAD, space="PSUM") as ps_y:
            t_w = [None]
            cb_w = [None]
            tps = {}

            def emit_T(q):
                if q % (GT // 2) == 0:
                    t_w[0] = twp.tile([L, GT, L], f16, name="t_w")
                    nc.scalar.dma_start(out=t_w[0], in_=t_all[q // (GT // 2)])
                y_ps = ps_y.tile([L, 2, BC], f32, name="y_ps")
                for par in range(2):
                    dl = 2 * q + par
                    nc.tensor.matmul(y_ps[:, par, :], lhsT=t_w[0][:, dl % GT, :],
                                     rhs=u_sb[:, dl, :], start=(par == 0),
                                     stop=False)
                tps[q] = y_ps

            def emit_cb(q):
                if (2 * q) % GQ == 0:
                    cb_w[0] = cbp.tile([128, GQ, L], f16, name="cb_w")
                    nc.sync.dma_start(out=cb_w[0], in_=cb_all[(2 * q) // GQ])
                h = xin[:, :, q, :]
                for par in range(2):
                    dl = 2 * q + par
                    nc.tensor.matmul(
                        tps[q][:, par, :],
                        lhsT=cb_w[0][:, dl % GQ, :],
                        rhs=h, start=False, stop=(par == 1))

            for q in range(AHEAD):
                emit_T(q)
            for q in range(64):
                emit_cb(q)
                if q < 32:
                    nc.scalar.copy(y_sb[:, 2 * q:2 * q + 2, :], tps[q])
                else:
                    cp(q, y_sb[:, 2 * q:2 * q + 2, :], tps[q])
                del tps[q]
                if q + AHEAD < 64:
                    emit_T(q + AHEAD)

        # ---- Phase E: PE-transpose y tiles + output projection ----
        # 16 tiles per group; 2-bank PSUM tiles halve per-op copy overhead
        with tc.tile_pool(name="ytp", bufs=2) as ytp, \
             tc.tile_pool(name="lop", bufs=2) as lop, \
             tc.tile_pool(name="ps_t", bufs=2, space="PSUM") as ps_t, \
             tc.tile_pool(name="ps_o", bufs=2, space="PSUM") as ps_o:
            for g in range(BC // 16):
                tp = ps_t.tile([128, 16, L], f16, name="tp")
                for i in range(16):
                    nc.tensor.transpose(tp[:, i, :], y_sb[:, :, 16 * g + i],
                                        ident)
                yt = ytp.tile([128, 16, L], f16, name="yt")
                nc.vector.tensor_copy(yt[:, 0:8, :], tp[:, 0:8, :])
                nc.scalar.copy(yt[:, 8:16, :], tp[:, 8:16, :])
                lo_ps = ps_o.tile([128, 4, 2 * L], f32, name="lo_ps")
                for p in range(8):
                    nc.tensor.matmul(
                        lo_ps[64 * (p % 2):64 * (p % 2) + 64, p // 2, :],
                        lhsT=w2_sb, rhs=yt[:, 2 * p:2 * p + 2, :],
                        start=True, stop=True,
                        tile_position=(0, 64) if p % 2 else None)
                lo = lop.tile([128, 4, 2 * L], f16, name="lo")
                nc.vector.tensor_copy(lo[:, 0:2, :], lo_ps[:, 0:2, :])
                nc.scalar.copy(lo[:, 2:4, :], lo_ps[:, 2:4, :])
                base = g * 16 * L
                oap = out_t[:, base:base + 16 * L].rearrange(
                    "v (s x) -> v s x", s=4)
                eng = (nc.gpsimd, nc.sync, nc.scalar)[g % 3]
                eng.dma_start(out=oap[:, :, 0:2 * L], in_=lo[0:64])
                eng.dma_start(out=oap[:, :, 2 * L:4 * L], in_=lo[64:128])


def _build_nc():
    import concourse.tile as tile
    from concourse import bacc, mybir
    from concourse.masks import make_identity

    nc = bacc.Bacc(trn_type="TRN2", target_bir_lowering=False, debug=False)
    _emit_kernel(nc, tile, mybir, make_identity)
    nc.compile()
    return nc


_NC_CACHE = None


def kernel(x, emb, log_neg_A, B, C, D, log_dt, W_out, b_out):
    global LAST_RESULTS, _NC_CACHE
    from concourse.bass_utils import run_bass_kernel_spmd

    x = np.asarray(x).astype(np.int64)
    emb = np.asarray(emb, np.float32)
    log_neg_A = np.asarray(log_neg_A, np.float32)
    B_in = np.asarray(B, np.float32)
    C = np.asarray(C, np.float32)
    D_in = np.asarray(D, np.float32)
    log_dt = np.asarray(log_dt, np.float32)
    W_out = np.asarray(W_out, np.float32)
    b_out = np.asarray(b_out, np.float32)

    T, E, Cb, P = _precompute_host(emb, log_neg_A, B_in, C, D_in, log_dt, W_out)

    # u = emb[x] in fp16; tile order t = b*NCH + c
    emb16 = emb.astype(np.float16)
    u_full = emb16[x].reshape(BATCH, NCH, L, D_MODEL)

    in_maps = []
    for core in range(NCORES):
        ds = slice(core * DPC, (core + 1) * DPC)
        # u_in: [j, dl, t=(b, c)]
        uc = np.ascontiguousarray(
            u_full[:, :, :, ds].transpose(2, 3, 0, 1).reshape(L, DPC, BC))
        # pmul layout [p=(par,n), q, c]: p = 64*par + n, d = 2*q + par
        Pc = P[ds].reshape(64, 2, N_STATE).transpose(1, 2, 0).reshape(128, 64)
        pm = np.broadcast_to(Pc[:, :, None], (128, 64, NCH)).copy()
        pm[:, :, 0] = 0.0
        # t_all: [DPC,L,L] -> [DPC/GT, L, GT, L] (ch groups, j-major)
        Tc = np.ascontiguousarray(
            T[ds].reshape(DPC // GT, GT, L, L).transpose(0, 2, 1, 3))
        # e_all: [DPC,L,N] -> [DPC/GE, L, GE, N]
        Ec = np.ascontiguousarray(
            E[ds].reshape(DPC // GE, GE, L, N_STATE).transpose(0, 2, 1, 3))
        # cb_all: [DPC,N,L] zero-padded to [DPC,128,L]: rows 64*(d%2)+n
        Cbpad = np.zeros((DPC, 128, L), np.float16)
        for par in range(2):
            Cbpad[par::2, 64 * par:64 * (par + 1), :] = Cb[ds][par::2]
        Cbc = np.ascontiguousarray(
            Cbpad.reshape(DPC // GQ, GQ, 128, L).transpose(0, 2, 1, 3))
        in_maps.append({
            "u_in": uc,
            "t_all": Tc,
            "e_all": Ec,
            "cb_all": Cbc,
            "pmul": pm.astype(np.float16),
            "w2": np.ascontiguousarray(W_out[ds]).astype(np.float16),
        })

    if _NC_CACHE is None:
        _NC_CACHE = _build_nc()
    nc = _NC_CACHE

    trace = bool(int(os.environ.get("BASS_TRACE", "0") or "0"))
    LAST_RESULTS = run_bass_kernel_spmd(
        nc, in_maps, core_ids=list(range(NCORES)), trace=trace)

    logitsT = np.zeros((VOCAB, BC * L), np.float64)
    for r in LAST_RESULTS.results:
        logitsT += r["out_t"].astype(np.float64)
    # columns tok = (b*NCH + c)*L + i
    out = logitsT.T.reshape(BATCH, NCH, L, VOCAB).reshape(BATCH, SEQ, VOCAB)
    return (out + b_out.astype(np.float64)).astype(np.float32)
